# revision 12
# baseline (speedup 1.0000x reference)
"""Trainium2 Bass kernel for nn_CoeffNet (equivariant GNN message passing).

Sharding: edges bucketed by destination-node block (8 blocks of 16384 nodes,
one per core) and sorted by dst; scatter is purely local per core; updated
node-feature blocks are exchanged between the two message-passing steps with
an on-chip AllGather. Small weights travel as 1/8 slices per core and are
reassembled on-chip by a 674 KB AllGather; constant rows (embed bias, rad
ones, Ysh l=0) are memset on device instead of transferred.

Inputs are packed into two consolidated per-core blobs (one f32, one
int32). The dedupe 0/1 matrix S (8.5 MB/core as f32 in the original
formulation) is reconstructed on device from a 66 KB column-index vector
via an iota/is_equal compare. The value path stays f32 end to end: the
equivariant gate (sign of the scalar channel) has gate margins down to
~1e-7 on this input, so quantizing x/rad/Ysh to fp16 or even 24-bit fixed
point flips gates and breaks correctness (verified by host simulation).
Only the final output is fp16 (no feedback).

Execution + measurement: the kernel runs via bass_utils.run_bass_kernel_spmd
on cores 0-7 with trace=True, which wraps the NEFF execution in NRT/NTFF
profiling (neuron-profile) and returns the hardware execution time of the
NEFF as measured on device, max over the 8 cores. LAST_EXEC_NS is that
neuron-profiled device execution time. (The agent image lacks
antenv.axon_hooks, so an equivalent ctypes NTFF hook against
libaxon_pjrt.so is installed first; the artifact-bucket upload is stubbed
to a local path since this container has no artifact store.)

Per core, one NEFF:
  embed (PE, per-tile stationary xdT chunks)  -> x0 block -> AllGather
  2x message-pass step:
      per 128-edge tile: indirect-DMA gather of xs rows from the gathered
      full feature table; coef = radT.T @ Wstack (PE); u' = YshT.T @ Gexp
      (PE); tensor product on DVE (triple-grouped wide ops); in-tile dedupe
      matmul (St one-hot built on DVE from colidx); indirect scatter-add
      into the residual staging block (rows globally unique by host
      construction).
      AllGather staging -> full table (before each step).
  final: per-tile PE transposes, W1 (PE), equivariant gate, W2 (PE) -> outT
  (fp16).
Host does: index bucketing/sorting/padding, Ysh/rad basis evaluation,
weight reshaping, blob packing, output concat+transpose.
"""
import numpy as np
from contextlib import ExitStack

E = 131072
FIN = 16
F = 32
B = 32
NCORES = 8
NB = E // NCORES
P = 128
SLC = [(0, 1), (1, 4), (4, 9)]
CENTERS = np.linspace(0.0, 4.0, B).astype(np.float32)
COLPAD = 500.0  # colidx sentinel for invalid edges (exact in fp16)

C0 = 0.28209479177387814
C1 = 0.4886025119029199
C2 = 1.0925484305920792
C20 = 0.31539156525252005
C22 = 0.5462742152960396


def _sh9_np(u):
    x, y, z = u[..., 0], u[..., 1], u[..., 2]
    return np.stack([
        np.full_like(x, C0), C1 * y, C1 * z, C1 * x,
        C2 * x * y, C2 * y * z, C20 * (3.0 * z * z - 1.0), C2 * x * z,
        C22 * (x * x - y * y)
    ], axis=-1)


def _gaunt_np():
    ct, w = np.polynomial.legendre.leggauss(8)
    phi = np.arange(16) * (2.0 * np.pi / 16.0)
    st = np.sqrt(1.0 - ct**2)
    X = (st[:, None] * np.cos(phi)[None, :]).ravel()
    Yc = (st[:, None] * np.sin(phi)[None, :]).ravel()
    Z = np.repeat(ct, 16)
    wq = np.repeat(w, 16) * (2.0 * np.pi / 16.0)
    Yg = _sh9_np(np.stack([X, Yc, Z], axis=-1))
    return np.einsum('qa,qb,qc,q->abc', Yg, Yg, Yg, wq)


GAUNT = _gaunt_np().astype(np.float32)
ALLOWED = [(l1, l2, l3)
           for l1 in range(3) for l2 in range(3) for l3 in range(3)
           if np.abs(GAUNT[SLC[l1][0]:SLC[l1][1], SLC[l2][0]:SLC[l2][1],
                          SLC[l3][0]:SLC[l3][1]]).max() > 1e-8]
NTRI = len(ALLOWED)  # 11

# Packed nonzero-term layout: per (t, c) only the a's with a nonzero Gaunt
# column survive, stored as strided runs (start, stride, count) so pi-mult
# APs stay regular. 115 dense terms -> 79 nonzero.
def _build_tc_info():
    info = []  # (t_idx, c_local, runs=[(s, d, n, gcol_off)])
    off = 0
    for t_idx, (l1, l2, l3) in enumerate(ALLOWED):
        (a0, a1), (b0, b1), (c0, c1) = SLC[l1], SLC[l2], SLC[l3]
        for c in range(c0, c1):
            nz = [a - a0 for a in range(a0, a1)
                  if np.abs(GAUNT[a, b0:b1, c]).max() > 1e-8]
            assert nz, (t_idx, c)
            runs = []
            i = 0
            while i < len(nz):
                if i + 1 < len(nz):
                    d = nz[i + 1] - nz[i]
                    j = i + 1
                    while j + 1 < len(nz) and nz[j + 1] - nz[j] == d:
                        j += 1
                else:
                    d, j = 1, i
                runs.append((nz[i], max(d, 1), j - i + 1, off))
                off += j - i + 1
                i = j + 1
            info.append((t_idx, c - c0, runs))
    return info, off


TC_INFO, NTERM = _build_tc_info()  # NTERM = 79 packed columns
NUP = -(-NTERM // 4) * 4


def build_gexp():
    M = np.zeros((9, NUP), np.float32)
    for t_idx, c_local, runs in TC_INFO:
        l1, l2, l3 = ALLOWED[t_idx]
        (a0, a1), (b0, b1), (c0, c1) = SLC[l1], SLC[l2], SLC[l3]
        c = c0 + c_local
        for (s, d, n, off) in runs:
            for ei in range(n):
                a = a0 + s + ei * d
                M[b0:b1, off + ei] = GAUNT[a, b0:b1, c]
    return M


def build_embed_mat(W, b, fin, fout):
    M = np.zeros((9 * fin + 1, 9 * fout), np.float32)
    for l, (s0, s1) in enumerate(SLC):
        for c in range(s0, s1):
            M[c * fin:(c + 1) * fin, c * fout:(c + 1) * fout] = W[l]
    M[9 * fin, 0:fout] = np.asarray(b).reshape(-1)
    return M


def build_wstack(Wmp_s, bmp_s):
    M = np.zeros((B + 1, NTRI * F), np.float32)
    for t_idx, (l1, l2, l3) in enumerate(ALLOWED):
        M[:B, t_idx * F:(t_idx + 1) * F] = Wmp_s[l1, l2, l3]
        M[B, t_idx * F:(t_idx + 1) * F] = bmp_s[l1, l2, l3]
    return M


def prep_cores(coords, dst_idx, src_idx):
    rel = coords[dst_idx] - coords[src_idx]
    r = np.sqrt(np.sum(rel * rel, axis=-1) + 1e-8)
    u = rel / r[:, None]
    Ysh = _sh9_np(u).astype(np.float32)
    rad = np.exp(-(r[:, None] - CENTERS[None, :])**2).astype(np.float32)
    rad1 = np.concatenate([rad, np.ones((E, 1), np.float32)], 1)

    cores = []
    for k in range(NCORES):
        e_ids = np.nonzero((dst_idx // NB) == k)[0]
        order = np.argsort(dst_idx[e_ids], kind='stable')
        e_ids = e_ids[order]
        dloc = dst_idx[e_ids] - k * NB
        out_src, out_dst, out_eid = [], [], []
        cur = 0
        i, n = 0, len(e_ids)
        while i < n:
            j = i
            while j < n and dloc[j] == dloc[i]:
                j += 1
            glen = j - i
            assert glen <= P, "dst in-degree exceeds one tile"
            room = (-cur) % P
            if room != 0 and glen > room:
                out_src += [0] * room
                out_dst += [-1] * room
                out_eid += [-1] * room
                cur += room
            out_src += [int(v) for v in src_idx[e_ids[i:j]]]
            out_dst += [int(dloc[i])] * glen
            out_eid += [int(v) for v in e_ids[i:j]]
            cur += glen
            i = j
        room = (-cur) % P
        out_src += [0] * room
        out_dst += [-1] * room
        out_eid += [-1] * room
        cores.append(dict(src=np.array(out_src, np.int64),
                          dst=np.array(out_dst, np.int64),
                          eid=np.array(out_eid, np.int64)))

    cap = max(len(c['src']) for c in cores)
    cap = -(-cap // P) * P
    ntiles = cap // P
    # chunk size: prefer ~12-16 tiles; pad ntiles so a divisor exists
    cht = 0
    for ct in (13, 16, 12, 14, 15, 11, 10, 9, 8):
        if ntiles % ct == 0:
            cht = ct
            break
    if cht == 0:
        cht = 13
        ntiles = -(-ntiles // cht) * cht
        cap = ntiles * P
    for c in cores:
        pad = cap - len(c['src'])
        c['src'] = np.concatenate([c['src'], np.zeros(pad, np.int64)])
        c['dst'] = np.concatenate([c['dst'], -np.ones(pad, np.int64)])
        c['eid'] = np.concatenate([c['eid'], -np.ones(pad, np.int64)])
        valid = c['eid'] >= 0
        YshT = np.zeros((9, cap), np.float32)
        radT = np.zeros((B + 1, cap), np.float32)
        YshT[:, valid] = Ysh[c['eid'][valid]].T
        radT[:, valid] = rad1[c['eid'][valid]].T
        c['YshT'] = YshT
        c['radT'] = radT
        c['srcidx'] = c['src'].astype(np.int32).reshape(ntiles, P)
        colidx = np.full((ntiles, P), COLPAD, np.float32)
        sidx = np.full((ntiles, P), NB, np.int32)
        d = c['dst']
        for t in range(ntiles):
            dt = d[t * P:(t + 1) * P]
            first = {}
            for e2 in range(P):
                if dt[e2] < 0:
                    continue
                if dt[e2] not in first:
                    first[dt[e2]] = e2
                    sidx[t, e2] = dt[e2]
                colidx[t, e2] = first[dt[e2]]
        c['colidx'] = colidx
        c['sidx'] = sidx
        # remap gather indices to the slab-major xfull layout:
        # global node g = j*NB + (s*2048 + r)  ->  s*(NCORES*2048) + j*2048 + r
        g = c['srcidx'].astype(np.int64)
        j = g // NB
        rl = g % NB
        c['srcidx'] = (
            (rl // 2048) * (NCORES * 2048) + j * 2048 + (rl % 2048)
        ).astype(np.int32)
    return cores, cap, ntiles, cht


def weights_layout():
    """(name -> (offset, rows, cols)) for the weights region of the per-core
    blob (full weights on every core; no on-chip reassembly)."""
    lay = {}
    off = 0
    for name, rows, cols in (
            ("M0a", P, 288), ("M0b", 17, 288),
            ("Wstk", B + 1, 2 * NTRI * F), ("Gexp", 9, NUP),
            ("IotaF", P, P), ("M1e", 97, 3 * 288), ("M2e", 97, 3 * 9)):
        lay[name] = (off, rows, cols)
        off += rows * cols
    return lay, off


def blob_layout(cap, ntiles):
    """(name -> (offset, rows, cols)) for the per-core f32 blob.  Constant
    rows (xdT2 bias row, rad ones row, Ysh l=0 row) are memset on device."""
    _, wtot = weights_layout()
    lay = {}
    off = 0
    for name, rows, cols in (
            ("xdT1", P, NB), ("xdT2", 16, NB),
            ("radT", B, cap), ("YshT", 8, cap),
            ("colidx", P, ntiles), ("wts", 1, wtot)):
        lay[name] = (off, rows, cols)
        off += rows * cols
    return lay, off


def build_program(cap, ntiles, cht):
    import concourse.bass as bass
    import concourse.bacc as bacc
    import concourse.mybir as mybir
    import concourse.tile as tile
    from concourse.masks import make_identity
    f32 = mybir.dt.float32
    f16 = mybir.dt.float16
    i32 = mybir.dt.int32
    AOP = mybir.AluOpType
    AP = bass.AP

    T = cht
    nchunks = ntiles // T
    layf, totf = blob_layout(cap, ntiles)
    wlay, wtot = weights_layout()

    nc = bacc.Bacc("TRN2", target_bir_lowering=False, debug=False,
                   num_devices=NCORES)

    blobf = nc.dram_tensor("blobf", [totf], f32, kind="ExternalInput").ap()
    blobi = nc.dram_tensor("blobi", [P, 2 * ntiles], i32,
                           kind="ExternalInput").ap()
    outG = nc.dram_tensor("outG", [9, NB], f16, kind="ExternalOutput").ap()

    def vf(name, coloff=0, ncols=None):
        off, rows, cols = layf[name]
        if ncols is None:
            ncols = cols - coloff
        return AP(blobf.tensor, off + coloff, [[cols, rows], [1, ncols]])

    def bc(ap, lvl, n):
        """insert a [0, n] broadcast level at free position lvl (0-based
        after partition dim)"""
        raw = list(ap.ap)
        raw.insert(1 + lvl, [0, n])
        return AP(ap.tensor, ap.offset, raw)

    NEMB = NB // 2048  # embed chunks == sub-AllGather slabs

    with tile.TileContext(nc) as tc, ExitStack() as ctx:
        dram = ctx.enter_context(tc.tile_pool(name="dram", bufs=1,
                                              space="DRAM"))
        const = ctx.enter_context(tc.tile_pool(name="const", bufs=1))

        staging = dram.tile([NB + 1, 288], f32)
        # per-slab embed outputs feed slab-wise sub-AllGathers that overlap
        # with the remaining embed compute (fine-grained deps need separate
        # tiles; a single staging tensor would serialize on whole-tensor deps)
        stagingE = [dram.tile([2048, 288], f32, name=f"stgE{s}")
                    for s in range(NEMB)]
        # Local (non-Shared) output: Shared DRAM only allows a single
        # writing instruction, and we write it with NEMB sub-AllGathers.
        xfull0 = dram.tile([NCORES * NB, 288], f32)
        xfull1 = dram.tile([NCORES * NB, 288], f32)
        myoutT = dram.tile([9, NB], f16)

        wts = vf("wts")

        def wview(name):
            off, rows, cols = wlay[name]
            return AP(wts.tensor, wts.offset + off,
                      [[cols, rows], [1, cols]])

        M0a_s = const.tile([P, 288], f32)
        nc.sync.dma_start(out=M0a_s[:], in_=wview("M0a"))
        M0b_s = const.tile([17, 288], f32)
        nc.sync.dma_start(out=M0b_s[:], in_=wview("M0b"))
        Wstk_s = const.tile([B + 1, 2 * NTRI * F], f32)
        nc.sync.dma_start(out=Wstk_s[:], in_=wview("Wstk"))
        Gexp_s = const.tile([9, NUP], f32)
        nc.sync.dma_start(out=Gexp_s[:], in_=wview("Gexp"))
        M1e_s = const.tile([97, 3 * 288], f32)
        nc.sync.dma_start(out=M1e_s[:], in_=wview("M1e"))
        M2e_s = const.tile([97, 3 * 9], f32)
        nc.sync.dma_start(out=M2e_s[:], in_=wview("M2e"))
        IotaF = const.tile([P, P], f32)
        nc.sync.dma_start(out=IotaF[:], in_=wview("IotaF"))
        ident = const.tile([P, P], f32)
        make_identity(nc, ident[:])
        srcidx_s = const.tile([P, ntiles], i32)
        nc.sync.dma_start(out=srcidx_s[:], in_=blobi[:, 0:ntiles])
        sidx_s = const.tile([P, ntiles], i32)
        nc.sync.dma_start(out=sidx_s[:], in_=blobi[:, ntiles:2 * ntiles])
        colidx_f = const.tile([P, ntiles], f32)
        nc.sync.dma_start(out=colidx_f[:], in_=vf("colidx"))

        # ---- embed (slab-wise, each slab's sub-AllGather overlaps the
        # remaining embed compute) ----
        x2cs = [const.tile([17, 2048], f32, name=f"x2c{i}") for i in range(2)]
        for t in x2cs:
            nc.vector.memset(t[0:1, :], 1.0)
        with tc.spectator_scope("embed"), \
             tc.tile_pool(name="emb", bufs=2) as emb, \
             tc.tile_pool(name="emb3", bufs=3) as emb3, \
             tc.tile_pool(name="psE", bufs=2, space="PSUM") as ps:
            for ch in range(NEMB):
                x1c = emb.tile([P, 2048], f32, tag="x1c")
                nc.sync.dma_start(out=x1c[:],
                                  in_=vf("xdT1", ch * 2048, 2048))
                x2c = x2cs[ch % 2]
                nc.sync.dma_start(out=x2c[1:17, :],
                                  in_=vf("xdT2", ch * 2048, 2048))
                for ti in range(16):
                    x0p = ps.tile([P, 288], f32, tag="x0p")
                    sl = slice(ti * P, (ti + 1) * P)
                    nc.tensor.matmul(out=x0p[:], lhsT=x1c[:, sl],
                                     rhs=M0a_s[:], start=True, stop=False)
                    nc.tensor.matmul(out=x0p[:], lhsT=x2c[:, sl],
                                     rhs=M0b_s[:], start=False, stop=True)
                    x0s = emb3.tile([P, 288], f32, tag="x0s")
                    nc.scalar.copy(out=x0s[:], in_=x0p[:])
                    node0 = ch * 2048 + ti * P
                    nc.sync.dma_start(out=staging[node0:node0 + P, :],
                                      in_=x0s[:])
                    nc.sync.dma_start(out=stagingE[ch][ti * P:(ti + 1) * P, :],
                                      in_=x0s[:])
                # sub-AllGather this slab into its contiguous slab-major
                # block of xfull0 (row layout: slab, core, local-row; the
                # host remaps srcidx to match). Collective outputs must be
                # contiguous (BIR verifier) - hence slab-major, not a
                # strided row-block view.
                g0 = ch * (NCORES * 2048)
                nc.gpsimd.collective_compute(
                    "AllGather", AOP.bypass,
                    replica_groups=[list(range(NCORES))],
                    ins=[stagingE[ch][:].opt()],
                    outs=[xfull0[g0:g0 + NCORES * 2048, :].opt()])

        # ---- message passing ----
        radcs = [const.tile([B + 1, T * P], f32, name=f"radc{i}")
                 for i in range(2)]
        yshcs = [const.tile([9, T * P], f32, name=f"yshc{i}")
                 for i in range(2)]
        for t in radcs:
            nc.vector.memset(t[B:B + 1, :], 1.0)
        for t in yshcs:
            nc.vector.memset(t[0:1, :], C0)
        with tc.tile_pool(name="mp", bufs=2) as mp, \
             tc.tile_pool(name="mp3", bufs=3) as mp3, \
             tc.tile_pool(name="psM", bufs=1, space="PSUM") as ps, \
             tc.tile_pool(name="psC", bufs=2, space="PSUM") as psc, \
             tc.tile_pool(name="psD", bufs=1, space="PSUM") as psd:
            for step in range(2):
                xfull = xfull0 if step == 0 else xfull1
                if step == 1:
                    for s in range(NEMB):
                        g0 = s * (NCORES * 2048)
                        nc.gpsimd.collective_compute(
                            "AllGather", AOP.bypass,
                            replica_groups=[list(range(NCORES))],
                            ins=[staging[s * 2048:(s + 1) * 2048, :].opt()],
                            outs=[xfull[g0:g0 + NCORES * 2048, :].opt()])
                wof = step * NTRI * F
                mpscope = tc.spectator_scope(f"mp{step}")
                mpscope.__enter__()
                for chn in range(nchunks):
                    t0 = chn * T
                    xs = mp.tile([P, T, 288], f32, tag="xs")
                    coef = mp.tile([P, T, NTRI * F], f32, tag="coef")
                    up = mp.tile([P, T, NUP], f32, tag="up")
                    msg = mp.tile([P, T, 288], f32, tag="msg")
                    radc = radcs[chn % 2]
                    nc.sync.dma_start(out=radc[0:B, :],
                                      in_=vf("radT", t0 * P, T * P))
                    yshc = yshcs[chn % 2]
                    nc.sync.dma_start(out=yshc[1:9, :],
                                      in_=vf("YshT", t0 * P, T * P))
                    for ti in range(T):
                        gt = t0 + ti
                        nc.gpsimd.indirect_dma_start(
                            out=xs[:, ti, :], out_offset=None,
                            in_=xfull[:],
                            in_offset=bass.IndirectOffsetOnAxis(
                                ap=srcidx_s[:, gt:gt + 1], axis=0))
                        cu = psc.tile([P, NTRI * F + NUP], f32, tag="cu")
                        nc.tensor.matmul(
                            out=cu[:, 0:NTRI * F],
                            lhsT=radc[:, ti * P:(ti + 1) * P],
                            rhs=Wstk_s[:, wof:wof + NTRI * F],
                            start=True, stop=True)
                        nc.scalar.copy(out=coef[:, ti, :],
                                       in_=cu[:, 0:NTRI * F])
                        nc.tensor.matmul(
                            out=cu[:, NTRI * F:NTRI * F + NUP],
                            lhsT=yshc[:, ti * P:(ti + 1) * P],
                            rhs=Gexp_s[:], start=True, stop=True)
                        nc.scalar.copy(out=up[:, ti, :],
                                       in_=cu[:, NTRI * F:NTRI * F + NUP])
                    # tensor product (Gaunt-sparse, strided runs).
                    # Term accumulation is split across engines: degrees
                    # accumulate on TensorE (identity matmuls into PSUM,
                    # start=first/stop=last per c-block) in two waves;
                    # wave B reuses the msgp slot (same tag, bufs=1) after
                    # wave A's ACT drains release it (WAR via pool dep).
                    for wave, clo, chi in ((0, 0, 5), (1, 5, 9)):
                      NC_W = chi - clo
                      wave_info = [
                          (t_idx, c_local, runs)
                          for t_idx, c_local, runs in TC_INFO
                          if clo <= SLC[ALLOWED[t_idx][2]][0] + c_local < chi]
                      last_op = {}
                      for t_idx, c_local, runs in wave_info:
                        cgl = SLC[ALLOWED[t_idx][2]][0] + c_local
                        for ri, (s, d, n, off) in enumerate(runs):
                            for ai in range(n):
                                last_op[cgl] = (t_idx, ri, ai)
                      msgp = ps.tile([P, NC_W, T * F], f32, tag="msgp",
                                     padded_shape=[P, NC_W, 512],
                                     name=f"msgp{wave}")
                      first_c = [True] * 9
                      qcache = {}
                      for t_idx, c_local, runs in wave_info:
                        l1, l2, l3 = ALLOWED[t_idx]
                        (a0, a1), (c0, c1) = SLC[l1], SLC[l3]
                        na = a1 - a0
                        if t_idx not in qcache:
                            q = mp.tile([P, T, 160], f32, tag="q",
                                        name=f"q{t_idx}")
                            nc.vector.tensor_tensor(
                                out=q[:, :, 0:na * F]
                                    .rearrange("p t (a f) -> p t a f", f=F),
                                in0=xs[:, :, a0 * F:a1 * F]
                                    .rearrange("p t (a f) -> p t a f", f=F),
                                in1=bc(coef[:, :, t_idx * F:(t_idx + 1) * F],
                                       1, na),
                                op=AOP.mult)
                            qcache[t_idx] = q
                        q = qcache[t_idx]
                        cgl = c0 + c_local
                        for ri, (s, d, n, off) in enumerate(runs):
                            pi = mp.tile([P, T, 160], f32, tag="pi")
                            qb = q[:, :, 0:160]
                            qrun = AP(qb.tensor, qb.offset + s * F,
                                      [qb.ap[0], qb.ap[1],
                                       [d * F, n], [1, F]])
                            nc.vector.tensor_tensor(
                                out=pi[:, :, 0:n * F]
                                    .rearrange("p t (a f) -> p t a f", f=F),
                                in0=qrun,
                                in1=bc(up[:, :, off:off + n], 2, F),
                                op=AOP.mult)
                            for ai in range(n):
                                nc.tensor.matmul(
                                    out=msgp[:, cgl - clo, :]
                                        .rearrange("p (t f) -> p t f",
                                                   f=F),
                                    lhsT=ident[:],
                                    rhs=pi[:, :, ai * F:(ai + 1) * F],
                                    start=first_c[cgl],
                                    stop=(last_op[cgl]
                                          == (t_idx, ri, ai)))
                                first_c[cgl] = False
                      # drain this wave's PE-accumulated c-blocks (ACT)
                      for cgl in range(clo, chi):
                        nc.scalar.copy(
                            out=msg[:, :, cgl * F:(cgl + 1) * F],
                            in_=msgp[:, cgl - clo, :]
                                .rearrange("p (t f) -> p t f", f=F))
                    # dedupe + scatter
                    for ti in range(T):
                        gt = t0 + ti
                        St = mp3.tile([P, P], f32, tag="St")
                        nc.vector.tensor_tensor(
                            out=St[:], in0=IotaF[:],
                            in1=bc(colidx_f[:, gt:gt + 1], 0, P),
                            op=AOP.is_equal)
                        ddp = psd.tile([P, 288], f32, tag="ddp")
                        nc.tensor.matmul(out=ddp[:], lhsT=St[:],
                                         rhs=msg[:, ti, :],
                                         start=True, stop=True)
                        scs = mp3.tile([P, 288], f32, tag="scs")
                        nc.scalar.copy(out=scs[:], in_=ddp[:])
                        nc.gpsimd.indirect_dma_start(
                            out=staging[:],
                            out_offset=bass.IndirectOffsetOnAxis(
                                ap=sidx_s[:, gt:gt + 1], axis=0),
                            in_=scs[:], in_offset=None,
                            compute_op=AOP.add)
                mpscope.__exit__(None, None, None)

        # ---- final layers ----
        with tc.tile_pool(name="fin", bufs=1) as fin, \
             tc.tile_pool(name="fin2", bufs=2) as fin2, \
             tc.tile_pool(name="fin3", bufs=3) as fin3, \
             tc.tile_pool(name="psF", bufs=2, space="PSUM") as ps:
            for ch in range(NB // 2048):
                x2T = fin.tile([97, 3, 16, P], f32, tag="x2T")
                nc.vector.memset(x2T[96:97, :, :, :], 1.0)
                for ti in range(16):
                    xt = fin3.tile([P, 288], f32, tag="xt")
                    node0 = ch * 2048 + ti * P
                    nc.sync.dma_start(out=xt[:],
                                      in_=staging[node0:node0 + P, :])
                    for kk in range(3):
                        tp = ps.tile([96, P], f32, tag="tp")
                        nc.tensor.transpose(
                            out=tp[:], in_=xt[:, kk * 96:(kk + 1) * 96],
                            identity=ident[:])
                        nc.scalar.copy(out=x2T[0:96, kk, ti, :], in_=tp[:])
                y1a = fin.tile([96, 3, 16, P], f32, tag="y1a")
                for m in range(3):
                    for ti in range(16):
                        y1p = ps.tile([96, P], f32, tag="y1p")
                        for kk in range(3):
                            nc.tensor.matmul(
                                out=y1p[:],
                                lhsT=M1e_s[:, m * 288 + kk * 96:
                                           m * 288 + (kk + 1) * 96],
                                rhs=x2T[:, kk, ti, :],
                                start=(kk == 0), stop=(kk == 2))
                        nc.scalar.copy(out=y1a[:, m, ti, :], in_=y1p[:])
                maskx = fin2.tile([96, 16, P], f32, tag="maskx")
                for cl in range(3):
                    nc.vector.tensor_scalar(
                        out=maskx[cl * F:(cl + 1) * F, :, :],
                        in0=y1a[0:F, 0, :, :],
                        scalar1=0.0, scalar2=None, op0=AOP.is_gt)
                y1g = fin.tile([97, 3, 16, P], f32, tag="y1g")
                nc.vector.memset(y1g[96:97, :, :, :], 1.0)
                for m in range(3):
                    nc.vector.tensor_tensor(
                        out=y1g[0:96, m, :, :], in0=y1a[:, m, :, :],
                        in1=maskx[:], op=AOP.mult)
                y2s = fin2.tile([9, 16, P], f16, tag="y2s")
                for ti in range(16):
                    y2p = ps.tile([9, P], f32, tag="y2p")
                    for m in range(3):
                        nc.tensor.matmul(
                            out=y2p[:], lhsT=M2e_s[:, m * 9:(m + 1) * 9],
                            rhs=y1g[:, m, ti, :],
                            start=(m == 0), stop=(m == 2))
                    nc.scalar.copy(out=y2s[:, ti, :], in_=y2p[:])
                nc.sync.dma_start(
                    out=myoutT[:, ch * 2048:(ch + 1) * 2048]
                        .rearrange("g (t p) -> g t p", p=P),
                    in_=y2s[:])

        nc.sync.dma_start(out=outG[:], in_=myoutT[:])

    return nc


LAST_EXEC_NS = None
_AXON_SO_PATH = "/opt/axon/libaxon_pjrt.so"


def _install_axon_profile_hooks():
    """Make bass_utils' axon NTFF-profiling path work on this image.

    run_bass_kernel_spmd(trace=True) under axon needs
    antenv.axon_hooks.get_axon_ntff_profile_hook(); the agent image ships
    without that module, so synthesize it with the slim ctypes hook
    (identical to trn_agent_boot.trn_boot._ntff_profile_via_ctypes). Also
    stub the artifact-bucket upload: no artifact store in this container.
    """
    import contextlib
    import ctypes
    import sys
    import types
    from concourse import bass_utils

    bass_utils.upload_artifacts = lambda tmpdir, **kw: "local://" + str(tmpdir)
    try:
        from antenv.axon_hooks import get_axon_ntff_profile_hook  # noqa: F401
        return
    except ImportError:
        pass

    lib = ctypes.CDLL(_AXON_SO_PATH)
    if not hasattr(lib, "axon_start_nrt_profile"):
        hook = None
    else:
        lib.axon_start_nrt_profile.argtypes = [
            ctypes.POINTER(ctypes.c_int64), ctypes.c_size_t]
        lib.axon_start_nrt_profile.restype = ctypes.c_int64
        lib.axon_stop_nrt_profile.argtypes = [ctypes.c_char_p]
        lib.axon_stop_nrt_profile.restype = ctypes.c_int64

        @contextlib.contextmanager
        def hook(output_dir, device_ids):
            import jax
            jax.devices()
            if device_ids:
                ids = (ctypes.c_int64 * len(device_ids))(*device_ids)
                rc = lib.axon_start_nrt_profile(ids, len(device_ids))
            else:
                rc = lib.axon_start_nrt_profile(None, 0)
            if rc != 0:
                raise RuntimeError(f"axon_start_nrt_profile rc={rc}")
            try:
                yield
            finally:
                n = lib.axon_stop_nrt_profile(str(output_dir).encode())
                if n < 0:
                    raise RuntimeError(f"axon_stop_nrt_profile rc={n}")

    mod = types.ModuleType("antenv.axon_hooks")
    mod.get_axon_ntff_profile_hook = lambda: hook
    mod.set_axon_ntff_profile_hook = lambda h: None
    sys.modules["antenv.axon_hooks"] = mod


def kernel(x_dftb, coords, dst_idx, src_idx, W0, b0, Wmp, bmp, W1, b1, W2, b2):

    x_dftb = np.asarray(x_dftb, np.float32)
    coords = np.asarray(coords, np.float32)
    dst_idx = np.asarray(dst_idx).astype(np.int64)
    src_idx = np.asarray(src_idx).astype(np.int64)

    cores, cap, ntiles, cht = prep_cores(coords, dst_idx, src_idx)
    layf, totf = blob_layout(cap, ntiles)

    M0 = build_embed_mat(np.asarray(W0), np.asarray(b0), FIN, F)
    M1 = build_embed_mat(np.asarray(W1), np.asarray(b1), F, F)
    M2 = build_embed_mat(np.asarray(W2), np.asarray(b2), F, 1)
    M0a_h = M0[0:P]
    # bias row first, then the 16 remaining weight rows (x2c row 0 is the
    # device-memset ones row)
    M0b_h = np.concatenate([M0[144:145], M0[P:144]], 0)

    def kchunks(M, ncols):
        out = np.zeros((3, 97, ncols), np.float32)
        for kk in range(3):
            out[kk, 0:96] = M[kk * 96:(kk + 1) * 96]
        out[2, 96] = M[288]
        return out

    M1e_h = kchunks(M1, 288).transpose(1, 0, 2).reshape(97, 3 * 288)
    M2e_h = kchunks(M2, 9).transpose(1, 0, 2).reshape(97, 3 * 9)
    Wstk_h = np.concatenate(
        [build_wstack(np.asarray(Wmp)[s], np.asarray(bmp)[s])
         for s in range(2)], 1)  # [33, 2*NTRI*F]
    Gexp_h = build_gexp()

    nc = build_program(cap, ntiles, cht)
    nc.finalize()

    wlay, wtot = weights_layout()
    wfull_h = np.zeros(wtot, np.float32)
    for name, arr in (("M0a", M0a_h), ("M0b", M0b_h), ("Wstk", Wstk_h),
                      ("Gexp", Gexp_h),
                      ("IotaF", np.broadcast_to(
                          np.arange(P, dtype=np.float32), (P, P))),
                      ("M1e", M1e_h), ("M2e", M2e_h)):
        off, rows, cols = wlay[name]
        wfull_h[off:off + rows * cols] = np.asarray(
            arr, np.float32).reshape(-1)

    xd = x_dftb.reshape(E, 144)
    in_maps = []
    for k in range(NCORES):
        c = cores[k]
        blk = xd[k * NB:(k + 1) * NB]
        bf = np.zeros(totf, np.float32)
        def putf(name, arr):
            off, rows, cols = layf[name]
            bf[off:off + rows * cols] = np.asarray(
                arr, np.float32).reshape(-1)
        putf("xdT1", blk[:, 0:P].T)
        putf("xdT2", blk[:, P:144].T)
        putf("radT", c['radT'][:B])
        putf("YshT", c['YshT'][1:9])
        putf("colidx", c['colidx'].T)
        putf("wts", wfull_h)
        bi = np.concatenate([c['srcidx'].T, c['sidx'].T], 1)
        in_maps.append({
            "blobf": bf,
            "blobi": np.ascontiguousarray(bi, np.int32),
        })

    import time
    from concourse import bass_utils

    _install_axon_profile_hooks()
    t0 = time.time()
    br = bass_utils.run_bass_kernel_spmd(
        nc, in_maps, core_ids=list(range(NCORES)), trace=True,
        trace_cores=list(range(NCORES)))
    wall_ns = int((time.time() - t0) * 1e9)
    global LAST_EXEC_NS, LAST_TRACE
    # exec_time_ns is the neuron-profiled NEFF execution time on device,
    # max over the 8 cores. Fall back to call wall time (which includes
    # tunnel transfer + jit compile) only if profiling was unavailable.
    LAST_EXEC_NS = br.exec_time_ns if br.exec_time_ns else wall_ns
    LAST_TRACE = br.instructions_and_trace[1] \
        if br.instructions_and_trace else None
    out = np.zeros((E, 1, 9, 1), np.float32)
    for k in range(NCORES):
        out[k * NB:(k + 1) * NB, 0, :, 0] = \
            br.results[k]["outG"].astype(np.float32).T  # [9, NB] f16
    return out



# revision 17
# speedup vs baseline: 1.0632x; 1.0632x over previous
"""Trainium2 Bass kernel for nn_CoeffNet (equivariant GNN message passing).

Sharding: edges bucketed by destination-node block (8 blocks of 16384 nodes,
one per core) and sorted by dst; scatter is purely local per core; updated
node-feature blocks are exchanged between the two message-passing steps with
an on-chip AllGather. Small weights travel as 1/8 slices per core and are
reassembled on-chip by a 674 KB AllGather; constant rows (embed bias, rad
ones, Ysh l=0) are memset on device instead of transferred.

Inputs are packed into two consolidated per-core blobs (one f32, one
int32). The dedupe 0/1 matrix S (8.5 MB/core as f32 in the original
formulation) is reconstructed on device from a 66 KB column-index vector
via an iota/is_equal compare. The value path stays f32 end to end: the
equivariant gate (sign of the scalar channel) has gate margins down to
~1e-7 on this input, so quantizing x/rad/Ysh to fp16 or even 24-bit fixed
point flips gates and breaks correctness (verified by host simulation).
Only the final output is fp16 (no feedback).

Execution + measurement: the kernel runs via bass_utils.run_bass_kernel_spmd
on cores 0-7 with trace=True, which wraps the NEFF execution in NRT/NTFF
profiling (neuron-profile) and returns the hardware execution time of the
NEFF as measured on device, max over the 8 cores. LAST_EXEC_NS is that
neuron-profiled device execution time. (The agent image lacks
antenv.axon_hooks, so an equivalent ctypes NTFF hook against
libaxon_pjrt.so is installed first; the artifact-bucket upload is stubbed
to a local path since this container has no artifact store.)

Per core, one NEFF:
  embed (PE, per-tile stationary xdT chunks)  -> x0 block -> AllGather
  2x message-pass step:
      per 128-edge tile: indirect-DMA gather of xs rows from the gathered
      full feature table; coef = radT.T @ Wstack (PE); u' = YshT.T @ Gexp
      (PE); tensor product on DVE (triple-grouped wide ops); in-tile dedupe
      matmul (St one-hot built on DVE from colidx); indirect scatter-add
      into the residual staging block (rows globally unique by host
      construction).
      AllGather staging -> full table (before each step).
  final: per-tile PE transposes, W1 (PE), equivariant gate, W2 (PE) -> outT
  (fp16).
Host does: index bucketing/sorting/padding, Ysh/rad basis evaluation,
weight reshaping, blob packing, output concat+transpose.
"""
import numpy as np
from contextlib import ExitStack

E = 131072
FIN = 16
F = 32
B = 32
NCORES = 8
NB = E // NCORES
P = 128
SLC = [(0, 1), (1, 4), (4, 9)]
CENTERS = np.linspace(0.0, 4.0, B).astype(np.float32)
COLPAD = 500.0  # colidx sentinel for invalid edges (exact in fp16)

C0 = 0.28209479177387814
C1 = 0.4886025119029199
C2 = 1.0925484305920792
C20 = 0.31539156525252005
C22 = 0.5462742152960396


def _sh9_np(u):
    x, y, z = u[..., 0], u[..., 1], u[..., 2]
    return np.stack([
        np.full_like(x, C0), C1 * y, C1 * z, C1 * x,
        C2 * x * y, C2 * y * z, C20 * (3.0 * z * z - 1.0), C2 * x * z,
        C22 * (x * x - y * y)
    ], axis=-1)


def _gaunt_np():
    ct, w = np.polynomial.legendre.leggauss(8)
    phi = np.arange(16) * (2.0 * np.pi / 16.0)
    st = np.sqrt(1.0 - ct**2)
    X = (st[:, None] * np.cos(phi)[None, :]).ravel()
    Yc = (st[:, None] * np.sin(phi)[None, :]).ravel()
    Z = np.repeat(ct, 16)
    wq = np.repeat(w, 16) * (2.0 * np.pi / 16.0)
    Yg = _sh9_np(np.stack([X, Yc, Z], axis=-1))
    return np.einsum('qa,qb,qc,q->abc', Yg, Yg, Yg, wq)


GAUNT = _gaunt_np().astype(np.float32)
ALLOWED = [(l1, l2, l3)
           for l1 in range(3) for l2 in range(3) for l3 in range(3)
           if np.abs(GAUNT[SLC[l1][0]:SLC[l1][1], SLC[l2][0]:SLC[l2][1],
                          SLC[l3][0]:SLC[l3][1]]).max() > 1e-8]
NTRI = len(ALLOWED)  # 11

# Packed nonzero-term layout: per (t, c) only the a's with a nonzero Gaunt
# column survive, stored as strided runs (start, stride, count) so pi-mult
# APs stay regular. 115 dense terms -> 79 nonzero.
def _build_tc_info():
    info = []  # (t_idx, c_local, runs=[(s, d, n, gcol_off)])
    off = 0
    for t_idx, (l1, l2, l3) in enumerate(ALLOWED):
        (a0, a1), (b0, b1), (c0, c1) = SLC[l1], SLC[l2], SLC[l3]
        for c in range(c0, c1):
            nz = [a - a0 for a in range(a0, a1)
                  if np.abs(GAUNT[a, b0:b1, c]).max() > 1e-8]
            assert nz, (t_idx, c)
            runs = []
            i = 0
            while i < len(nz):
                if i + 1 < len(nz):
                    d = nz[i + 1] - nz[i]
                    j = i + 1
                    while j + 1 < len(nz) and nz[j + 1] - nz[j] == d:
                        j += 1
                else:
                    d, j = 1, i
                runs.append((nz[i], max(d, 1), j - i + 1, off))
                off += j - i + 1
                i = j + 1
            info.append((t_idx, c - c0, runs))
    return info, off


TC_INFO, NTERM = _build_tc_info()  # NTERM = 79 packed columns
NUP = -(-NTERM // 4) * 4


def build_gexp():
    M = np.zeros((9, NUP), np.float32)
    for t_idx, c_local, runs in TC_INFO:
        l1, l2, l3 = ALLOWED[t_idx]
        (a0, a1), (b0, b1), (c0, c1) = SLC[l1], SLC[l2], SLC[l3]
        c = c0 + c_local
        for (s, d, n, off) in runs:
            for ei in range(n):
                a = a0 + s + ei * d
                M[b0:b1, off + ei] = GAUNT[a, b0:b1, c]
    return M


def build_embed_mat(W, b, fin, fout):
    M = np.zeros((9 * fin + 1, 9 * fout), np.float32)
    for l, (s0, s1) in enumerate(SLC):
        for c in range(s0, s1):
            M[c * fin:(c + 1) * fin, c * fout:(c + 1) * fout] = W[l]
    M[9 * fin, 0:fout] = np.asarray(b).reshape(-1)
    return M


def build_wstack(Wmp_s, bmp_s):
    M = np.zeros((B + 1, NTRI * F), np.float32)
    for t_idx, (l1, l2, l3) in enumerate(ALLOWED):
        M[:B, t_idx * F:(t_idx + 1) * F] = Wmp_s[l1, l2, l3]
        M[B, t_idx * F:(t_idx + 1) * F] = bmp_s[l1, l2, l3]
    return M


def prep_cores(coords, dst_idx, src_idx):
    """Slab-aligned edge layout: per core, edges are bucketed by dst slab
    (8 slabs of 2048 nodes), dst-sorted and group-padded within each slab,
    and every slab is padded to a common tile count Ts so tile->slab is
    compile-time (tile gt belongs to slab gt // Ts on every core)."""
    rel = coords[dst_idx] - coords[src_idx]
    r = np.sqrt(np.sum(rel * rel, axis=-1) + 1e-8)
    u = rel / r[:, None]
    Ysh = _sh9_np(u).astype(np.float32)
    rad = np.exp(-(r[:, None] - CENTERS[None, :])**2).astype(np.float32)
    rad1 = np.concatenate([rad, np.ones((E, 1), np.float32)], 1)
    NSLAB = NB // 2048
    HROWS = NCORES * (NB // 2)  # rows per half-table

    def build_stream(eids, dl):
        out_src, out_dst, out_eid = [], [], []
        cur, i, n = 0, 0, len(eids)
        while i < n:
            j = i
            while j < n and dl[j] == dl[i]:
                j += 1
            glen = j - i
            assert glen <= P, "dst in-degree exceeds one tile"
            room = (-cur) % P
            if room != 0 and glen > room:
                out_src += [0] * room
                out_dst += [-1] * room
                out_eid += [-1] * room
                cur += room
            out_src += [int(v) for v in src_idx[eids[i:j]]]
            out_dst += [int(dl[i])] * glen
            out_eid += [int(v) for v in eids[i:j]]
            cur += glen
            i = j
        return out_src, out_dst, out_eid

    streams = []
    for k in range(NCORES):
        e_ids = np.nonzero((dst_idx // NB) == k)[0]
        order = np.argsort(dst_idx[e_ids], kind='stable')
        e_ids = e_ids[order]
        dloc = dst_idx[e_ids] - k * NB
        row = []
        for s in range(NSLAB):
            m = (dloc // 2048) == s
            row.append(build_stream(e_ids[m], dloc[m] - s * 2048))
        streams.append(row)

    maxlen = max(len(st[0]) for row in streams for st in row)
    Ts = -(-maxlen // P)
    if Ts % 2:
        Ts += 1
    ntiles = NSLAB * Ts
    cap = ntiles * P
    cht = Ts // 2

    cores = []
    for k in range(NCORES):
        srca = np.zeros(cap, np.int64)
        dsta = np.full(cap, -1, np.int64)   # slab-local dst [0, 2048)
        eida = np.full(cap, -1, np.int64)
        for s in range(NSLAB):
            osrc, odst, oeid = streams[k][s]
            o0 = s * Ts * P
            srca[o0:o0 + len(osrc)] = osrc
            dsta[o0:o0 + len(osrc)] = odst
            eida[o0:o0 + len(osrc)] = oeid
        c = dict(src=srca, dst=dsta, eid=eida)
        valid = eida >= 0
        YshT = np.zeros((9, cap), np.float32)
        radT = np.zeros((B + 1, cap), np.float32)
        YshT[:, valid] = Ysh[eida[valid]].T
        radT[:, valid] = rad1[eida[valid]].T
        c['YshT'] = YshT
        c['radT'] = radT
        # gather indices split into lo/hi half-tables (row = owner*8192 +
        # local_row_within_half); out-of-half entries are OOB-skipped
        g = srca
        j = g // NB
        rl = g % NB
        OOB = 1 << 20
        lo = np.where(rl < NB // 2, j * (NB // 2) + rl, OOB)
        hi = np.where(rl >= NB // 2, j * (NB // 2) + (rl - NB // 2), OOB)
        c['srclo'] = lo.astype(np.int32).reshape(ntiles, P)
        c['srchi'] = hi.astype(np.int32).reshape(ntiles, P)
        # scatter targets the half-staging tensor of the tile's slab;
        # sidx is half-local ((s%4)*2048 + slab-local dst), trash = NB//2
        colidx = np.full((ntiles, P), COLPAD, np.float32)
        sidx = np.full((ntiles, P), NB // 2, np.int32)
        d = c['dst']
        for t in range(ntiles):
            s = t // Ts
            dt = d[t * P:(t + 1) * P]
            first = {}
            for e2 in range(P):
                if dt[e2] < 0:
                    continue
                if dt[e2] not in first:
                    first[dt[e2]] = e2
                    sidx[t, e2] = (s % 4) * 2048 + dt[e2]
                colidx[t, e2] = first[dt[e2]]
        c['colidx'] = colidx
        c['sidx'] = sidx
        cores.append(c)
    return cores, cap, ntiles, cht


def weights_layout():
    """(name -> (offset, rows, cols)) for the weights region of the per-core
    blob (full weights on every core; no on-chip reassembly)."""
    lay = {}
    off = 0
    for name, rows, cols in (
            ("M0a", P, 288), ("M0b", 17, 288),
            ("Wstk", B + 1, 2 * NTRI * F), ("Gexp", 9, NUP),
            ("IotaF", P, P), ("M1e", 97, 3 * 288), ("M2e", 97, 3 * 9)):
        lay[name] = (off, rows, cols)
        off += rows * cols
    return lay, off


def blob_layout(cap, ntiles):
    """(name -> (offset, rows, cols)) for the per-core f32 blob.  Constant
    rows (xdT2 bias row, rad ones row, Ysh l=0 row) are memset on device."""
    _, wtot = weights_layout()
    lay = {}
    off = 0
    for name, rows, cols in (
            ("xdT1", P, NB), ("xdT2", 16, NB),
            ("radT", B, cap), ("YshT", 8, cap),
            ("colidx", P, ntiles), ("wts", 1, wtot)):
        lay[name] = (off, rows, cols)
        off += rows * cols
    return lay, off


def build_program(cap, ntiles, cht):
    import concourse.bass as bass
    import concourse.bacc as bacc
    import concourse.mybir as mybir
    import concourse.tile as tile
    from concourse.masks import make_identity
    f32 = mybir.dt.float32
    f16 = mybir.dt.float16
    i32 = mybir.dt.int32
    AOP = mybir.AluOpType
    AP = bass.AP

    T = cht
    nchunks = ntiles // T
    layf, totf = blob_layout(cap, ntiles)
    wlay, wtot = weights_layout()

    nc = bacc.Bacc("TRN2", target_bir_lowering=False, debug=False,
                   num_devices=NCORES)

    blobf = nc.dram_tensor("blobf", [totf], f32, kind="ExternalInput").ap()
    blobi = nc.dram_tensor("blobi", [P, 3 * ntiles], i32,
                           kind="ExternalInput").ap()
    outG = nc.dram_tensor("outG", [9, NB], f16, kind="ExternalOutput").ap()

    def vf(name, coloff=0, ncols=None):
        off, rows, cols = layf[name]
        if ncols is None:
            ncols = cols - coloff
        return AP(blobf.tensor, off + coloff, [[cols, rows], [1, ncols]])

    def bc(ap, lvl, n):
        """insert a [0, n] broadcast level at free position lvl (0-based
        after partition dim)"""
        raw = list(ap.ap)
        raw.insert(1 + lvl, [0, n])
        return AP(ap.tensor, ap.offset, raw)

    NEMB = NB // 2048          # embed chunks == slabs
    Ts = ntiles // NEMB        # tiles per slab (T = Ts // 2)
    HROWS = NCORES * (NB // 2)  # rows per half gather-table
    HBC = HROWS - 1            # gather bounds check (greater => skipped)

    with tile.TileContext(nc) as tc, ExitStack() as ctx:
        dram = ctx.enter_context(tc.tile_pool(name="dram", bufs=1,
                                              space="DRAM"))
        const = ctx.enter_context(tc.tile_pool(name="const", bufs=1))

        # two half-staging tensors (half h = node rows [h*8192,(h+1)*8192)
        # + its own trash row 8192): scatters target the compile-time-known
        # half, so an AllGather of one half never false-serializes against
        # scatters into the other half
        stgh = [dram.tile([NB // 2 + 1, 288], f32, name=f"stg{h}")
                for h in range(2)]
        # two Shared half-tables per step (Shared output allows a single
        # writer only, and Shared-output collectives are ~2x faster than
        # Local-output ones); gathers do a lo+hi pair with OOB skip
        xf = [[dram.tile([HROWS, 288], f32, addr_space="Shared",
                         name=f"xf{st}{h}") for h in range(2)]
              for st in range(2)]
        myoutT = dram.tile([9, NB], f16)

        wts = vf("wts")

        def wview(name):
            off, rows, cols = wlay[name]
            return AP(wts.tensor, wts.offset + off,
                      [[cols, rows], [1, cols]])

        M0a_s = const.tile([P, 288], f32)
        nc.sync.dma_start(out=M0a_s[:], in_=wview("M0a"))
        M0b_s = const.tile([17, 288], f32)
        nc.sync.dma_start(out=M0b_s[:], in_=wview("M0b"))
        Wstk_s = const.tile([B + 1, 2 * NTRI * F], f32)
        nc.sync.dma_start(out=Wstk_s[:], in_=wview("Wstk"))
        Gexp_s = const.tile([9, NUP], f32)
        nc.sync.dma_start(out=Gexp_s[:], in_=wview("Gexp"))
        M1e_s = const.tile([97, 3 * 288], f32)
        nc.sync.dma_start(out=M1e_s[:], in_=wview("M1e"))
        M2e_s = const.tile([97, 3 * 9], f32)
        nc.sync.dma_start(out=M2e_s[:], in_=wview("M2e"))
        IotaF = const.tile([P, P], f32)
        nc.sync.dma_start(out=IotaF[:], in_=wview("IotaF"))
        ident = const.tile([P, P], f32)
        make_identity(nc, ident[:])
        srclo_s = const.tile([P, ntiles], i32)
        nc.sync.dma_start(out=srclo_s[:], in_=blobi[:, 0:ntiles])
        srchi_s = const.tile([P, ntiles], i32)
        nc.sync.dma_start(out=srchi_s[:], in_=blobi[:, ntiles:2 * ntiles])
        sidx_s = const.tile([P, ntiles], i32)
        nc.sync.dma_start(out=sidx_s[:], in_=blobi[:, 2 * ntiles:3 * ntiles])
        colidx_f = const.tile([P, ntiles], f32)
        nc.sync.dma_start(out=colidx_f[:], in_=vf("colidx"))

        def half_ag(step, h):
            nc.gpsimd.collective_compute(
                "AllGather", AOP.bypass,
                replica_groups=[list(range(NCORES))],
                ins=[stgh[h][0:NB // 2, :].opt()],
                outs=[xf[step][h][:].opt()])

        # ---- embed (half AllGathers overlap the remaining embed) ----
        x2cs = [const.tile([17, 2048], f32, name=f"x2c{i}") for i in range(2)]
        for t in x2cs:
            nc.vector.memset(t[0:1, :], 1.0)
        with tc.spectator_scope("embed"), \
             tc.tile_pool(name="emb", bufs=2) as emb, \
             tc.tile_pool(name="emb3", bufs=3) as emb3, \
             tc.tile_pool(name="psE", bufs=2, space="PSUM") as ps:
            for ch in range(NEMB):
                x1c = emb.tile([P, 2048], f32, tag="x1c")
                nc.sync.dma_start(out=x1c[:],
                                  in_=vf("xdT1", ch * 2048, 2048))
                x2c = x2cs[ch % 2]
                nc.sync.dma_start(out=x2c[1:17, :],
                                  in_=vf("xdT2", ch * 2048, 2048))
                for ti in range(16):
                    x0p = ps.tile([P, 288], f32, tag="x0p")
                    sl = slice(ti * P, (ti + 1) * P)
                    nc.tensor.matmul(out=x0p[:], lhsT=x1c[:, sl],
                                     rhs=M0a_s[:], start=True, stop=False)
                    nc.tensor.matmul(out=x0p[:], lhsT=x2c[:, sl],
                                     rhs=M0b_s[:], start=False, stop=True)
                    x0s = emb3.tile([P, 288], f32, tag="x0s")
                    nc.scalar.copy(out=x0s[:], in_=x0p[:])
                    node0 = ch * 2048 + ti * P
                    hh, loc = node0 // (NB // 2), node0 % (NB // 2)
                    nc.sync.dma_start(out=stgh[hh][loc:loc + P, :],
                                      in_=x0s[:])
                if ch == NEMB // 2 - 1:
                    half_ag(0, 0)
                elif ch == NEMB - 1:
                    half_ag(0, 1)

        # ---- message passing + final (final interleaved into step 1) ----
        radcs = [const.tile([B + 1, T * P], f32, name=f"radc{i}")
                 for i in range(2)]
        yshcs = [const.tile([9, T * P], f32, name=f"yshc{i}")
                 for i in range(2)]
        for t in radcs:
            nc.vector.memset(t[B:B + 1, :], 1.0)
        for t in yshcs:
            nc.vector.memset(t[0:1, :], C0)
        with tc.tile_pool(name="mp", bufs=2) as mp, \
             tc.tile_pool(name="mp3", bufs=3) as mp3, \
             tc.tile_pool(name="fin", bufs=1) as fin, \
             tc.tile_pool(name="fin2", bufs=2) as fin2, \
             tc.tile_pool(name="psM", bufs=1, space="PSUM") as ps, \
             tc.tile_pool(name="psC", bufs=2, space="PSUM") as psc, \
             tc.tile_pool(name="psD", bufs=1, space="PSUM") as psd, \
             tc.tile_pool(name="psF", bufs=1, space="PSUM") as psf:

            def emit_final(f):
                """W1 -> gate -> W2 for node rows [f*1024, (f+1)*1024)."""
                n0 = f * 1024
                hh, l0 = n0 // (NB // 2), n0 % (NB // 2)
                xtb = fin.tile([P, 8, 288], f32, tag="xtb")
                for ti in range(8):
                    nc.sync.dma_start(
                        out=xtb[:, ti, :],
                        in_=stgh[hh][l0 + ti * P:l0 + (ti + 1) * P, :])
                x2T = fin.tile([97, 3, 8, P], f32, tag="x2T")
                nc.vector.memset(x2T[96:97, :, :, :], 1.0)
                for kk in range(3):
                    for tb in range(2):
                        tp = psf.tile([96, 4, P], f32, tag="tp",
                                      name=f"tp{kk}{tb}")
                        for t4 in range(4):
                            ti = tb * 4 + t4
                            nc.tensor.transpose(
                                out=tp[:, t4, :],
                                in_=xtb[:, ti, kk * 96:(kk + 1) * 96],
                                identity=ident[:])
                        nc.scalar.copy(
                            out=x2T[0:96, kk, tb * 4:(tb + 1) * 4, :],
                            in_=tp[:])
                y1a = fin.tile([96, 3, 8, P], f32, tag="y1a")
                for m in range(3):
                    for tb in range(2):
                        y1p = psf.tile([96, 4, P], f32, tag="y1p")
                        for t4 in range(4):
                            ti = tb * 4 + t4
                            for kk in range(3):
                                nc.tensor.matmul(
                                    out=y1p[:, t4, :],
                                    lhsT=M1e_s[:, m * 288 + kk * 96:
                                               m * 288 + (kk + 1) * 96],
                                    rhs=x2T[:, kk, ti, :],
                                    start=(kk == 0), stop=(kk == 2))
                        nc.scalar.copy(
                            out=y1a[:, m, tb * 4:(tb + 1) * 4, :],
                            in_=y1p[:])
                maskx = fin2.tile([96, 8, P], f32, tag="maskx")
                for cl in range(3):
                    nc.vector.tensor_scalar(
                        out=maskx[cl * F:(cl + 1) * F, :, :],
                        in0=y1a[0:F, 0, :, :],
                        scalar1=0.0, scalar2=None, op0=AOP.is_gt)
                y1g = fin.tile([97, 3, 8, P], f32, tag="y1g")
                nc.vector.memset(y1g[96:97, :, :, :], 1.0)
                for m in range(3):
                    nc.vector.tensor_tensor(
                        out=y1g[0:96, m, :, :], in0=y1a[:, m, :, :],
                        in1=maskx[:], op=AOP.mult)
                y2s = fin2.tile([9, 8, P], f16, tag="y2s")
                for tb in range(2):
                    y2p = psf.tile([9, 4, P], f32, tag="tp",
                                   name=f"y2p{tb}")
                    for t4 in range(4):
                        ti = tb * 4 + t4
                        for m in range(3):
                            nc.tensor.matmul(
                                out=y2p[:, t4, :],
                                lhsT=M2e_s[:, m * 9:(m + 1) * 9],
                                rhs=y1g[:, m, ti, :],
                                start=(m == 0), stop=(m == 2))
                    nc.scalar.copy(out=y2s[:, tb * 4:(tb + 1) * 4, :],
                                   in_=y2p[:])
                nc.sync.dma_start(
                    out=myoutT[:, n0:n0 + 1024]
                        .rearrange("g (t p) -> g t p", p=P),
                    in_=y2s[:])

            for step in range(2):
                lo_t, hi_t = xf[step]
                wof = step * NTRI * F
                mpscope = tc.spectator_scope(f"mp{step}")
                mpscope.__enter__()
                for chn in range(nchunks):
                    t0 = chn * T
                    xs = mp.tile([P, T, 288], f32, tag="xs")
                    coef = mp.tile([P, T, NTRI * F], f32, tag="coef")
                    up = mp.tile([P, T, NUP], f32, tag="up")
                    msg = mp.tile([P, T, 288], f32, tag="msg")
                    radc = radcs[chn % 2]
                    nc.sync.dma_start(out=radc[0:B, :],
                                      in_=vf("radT", t0 * P, T * P))
                    yshc = yshcs[chn % 2]
                    nc.sync.dma_start(out=yshc[1:9, :],
                                      in_=vf("YshT", t0 * P, T * P))
                    for ti in range(T):
                        gt = t0 + ti
                        nc.gpsimd.indirect_dma_start(
                            out=xs[:, ti, :], out_offset=None,
                            in_=lo_t[:],
                            in_offset=bass.IndirectOffsetOnAxis(
                                ap=srclo_s[:, gt:gt + 1], axis=0),
                            bounds_check=HBC, oob_is_err=False)
                        nc.gpsimd.indirect_dma_start(
                            out=xs[:, ti, :], out_offset=None,
                            in_=hi_t[:],
                            in_offset=bass.IndirectOffsetOnAxis(
                                ap=srchi_s[:, gt:gt + 1], axis=0),
                            bounds_check=HBC, oob_is_err=False)
                        cu = psc.tile([P, NTRI * F + NUP], f32, tag="cu")
                        nc.tensor.matmul(
                            out=cu[:, 0:NTRI * F],
                            lhsT=radc[:, ti * P:(ti + 1) * P],
                            rhs=Wstk_s[:, wof:wof + NTRI * F],
                            start=True, stop=True)
                        nc.scalar.copy(out=coef[:, ti, :],
                                       in_=cu[:, 0:NTRI * F])
                        nc.tensor.matmul(
                            out=cu[:, NTRI * F:NTRI * F + NUP],
                            lhsT=yshc[:, ti * P:(ti + 1) * P],
                            rhs=Gexp_s[:], start=True, stop=True)
                        nc.scalar.copy(out=up[:, ti, :],
                                       in_=cu[:, NTRI * F:NTRI * F + NUP])
                    # tensor product (Gaunt-sparse, strided runs); c-degree
                    # accumulation on TensorE (identity matmuls into PSUM),
                    # three 3-c waves sharing one 3-bank PSUM slot
                    for wave, clo, chi in ((0, 0, 3), (1, 3, 6), (2, 6, 9)):
                      NC_W = chi - clo
                      wave_info = [
                          (t_idx, c_local, runs)
                          for t_idx, c_local, runs in TC_INFO
                          if clo <= SLC[ALLOWED[t_idx][2]][0] + c_local < chi]
                      last_op = {}
                      for t_idx, c_local, runs in wave_info:
                        cgl = SLC[ALLOWED[t_idx][2]][0] + c_local
                        for ri, (s, d, n, off) in enumerate(runs):
                            for ai in range(n):
                                last_op[cgl] = (t_idx, ri, ai)
                      msgp = ps.tile([P, NC_W, T * F], f32, tag="msgp",
                                     padded_shape=[P, NC_W, 512],
                                     name=f"msgp{wave}")
                      first_c = [True] * 9
                      qcache = {}
                      for t_idx, c_local, runs in wave_info:
                        l1, l2, l3 = ALLOWED[t_idx]
                        (a0, a1), (c0, c1) = SLC[l1], SLC[l3]
                        na = a1 - a0
                        if t_idx not in qcache:
                            q = mp.tile([P, T, 160], f32, tag="q",
                                        name=f"q{t_idx}")
                            nc.vector.tensor_tensor(
                                out=q[:, :, 0:na * F]
                                    .rearrange("p t (a f) -> p t a f", f=F),
                                in0=xs[:, :, a0 * F:a1 * F]
                                    .rearrange("p t (a f) -> p t a f", f=F),
                                in1=bc(coef[:, :, t_idx * F:(t_idx + 1) * F],
                                       1, na),
                                op=AOP.mult)
                            qcache[t_idx] = q
                        q = qcache[t_idx]
                        cgl = c0 + c_local
                        for ri, (s, d, n, off) in enumerate(runs):
                            pi = mp.tile([P, T, 160], f32, tag="pi")
                            qb = q[:, :, 0:160]
                            qrun = AP(qb.tensor, qb.offset + s * F,
                                      [qb.ap[0], qb.ap[1],
                                       [d * F, n], [1, F]])
                            nc.vector.tensor_tensor(
                                out=pi[:, :, 0:n * F]
                                    .rearrange("p t (a f) -> p t a f", f=F),
                                in0=qrun,
                                in1=bc(up[:, :, off:off + n], 2, F),
                                op=AOP.mult)
                            for ai in range(n):
                                nc.tensor.matmul(
                                    out=msgp[:, cgl - clo, :]
                                        .rearrange("p (t f) -> p t f",
                                                   f=F),
                                    lhsT=ident[:],
                                    rhs=pi[:, :, ai * F:(ai + 1) * F],
                                    start=first_c[cgl],
                                    stop=(last_op[cgl]
                                          == (t_idx, ri, ai)))
                                first_c[cgl] = False
                      for cgl in range(clo, chi):
                        nc.scalar.copy(
                            out=msg[:, :, cgl * F:(cgl + 1) * F],
                            in_=msgp[:, cgl - clo, :]
                                .rearrange("p (t f) -> p t f", f=F))
                    # dedupe + scatter (out view limited to the tile's slab
                    # tail so range-based deps stay slab-granular)
                    for ti in range(T):
                        gt = t0 + ti
                        slab = gt // Ts
                        St = mp3.tile([P, P], f32, tag="St")
                        nc.vector.tensor_tensor(
                            out=St[:], in0=IotaF[:],
                            in1=bc(colidx_f[:, gt:gt + 1], 0, P),
                            op=AOP.is_equal)
                        ddp = psd.tile([P, 288], f32, tag="ddp")
                        nc.tensor.matmul(out=ddp[:], lhsT=St[:],
                                         rhs=msg[:, ti, :],
                                         start=True, stop=True)
                        scs = mp3.tile([P, 288], f32, tag="scs")
                        nc.scalar.copy(out=scs[:], in_=ddp[:])
                        nc.gpsimd.indirect_dma_start(
                            out=stgh[slab // 4][:],
                            out_offset=bass.IndirectOffsetOnAxis(
                                ap=sidx_s[:, gt:gt + 1], axis=0),
                            in_=scs[:], in_offset=None,
                            compute_op=AOP.add)
                    if step == 0:
                        if chn == nchunks // 2 - 1:
                            half_ag(1, 0)
                        elif chn == nchunks - 1:
                            half_ag(1, 1)
                    else:
                        if chn % 2 == 1:
                            emit_final(chn - 1)
                            emit_final(chn)
                mpscope.__exit__(None, None, None)

        nc.sync.dma_start(out=outG[:], in_=myoutT[:])

    return nc


LAST_EXEC_NS = None
_AXON_SO_PATH = "/opt/axon/libaxon_pjrt.so"


def _install_axon_profile_hooks():
    """Make bass_utils' axon NTFF-profiling path work on this image.

    run_bass_kernel_spmd(trace=True) under axon needs
    antenv.axon_hooks.get_axon_ntff_profile_hook(); the agent image ships
    without that module, so synthesize it with the slim ctypes hook
    (identical to trn_agent_boot.trn_boot._ntff_profile_via_ctypes). Also
    stub the artifact-bucket upload: no artifact store in this container.
    """
    import contextlib
    import ctypes
    import sys
    import types
    from concourse import bass_utils

    bass_utils.upload_artifacts = lambda tmpdir, **kw: "local://" + str(tmpdir)
    try:
        from antenv.axon_hooks import get_axon_ntff_profile_hook  # noqa: F401
        return
    except ImportError:
        pass

    lib = ctypes.CDLL(_AXON_SO_PATH)
    if not hasattr(lib, "axon_start_nrt_profile"):
        hook = None
    else:
        lib.axon_start_nrt_profile.argtypes = [
            ctypes.POINTER(ctypes.c_int64), ctypes.c_size_t]
        lib.axon_start_nrt_profile.restype = ctypes.c_int64
        lib.axon_stop_nrt_profile.argtypes = [ctypes.c_char_p]
        lib.axon_stop_nrt_profile.restype = ctypes.c_int64

        @contextlib.contextmanager
        def hook(output_dir, device_ids):
            import jax
            jax.devices()
            if device_ids:
                ids = (ctypes.c_int64 * len(device_ids))(*device_ids)
                rc = lib.axon_start_nrt_profile(ids, len(device_ids))
            else:
                rc = lib.axon_start_nrt_profile(None, 0)
            if rc != 0:
                raise RuntimeError(f"axon_start_nrt_profile rc={rc}")
            try:
                yield
            finally:
                n = lib.axon_stop_nrt_profile(str(output_dir).encode())
                if n < 0:
                    raise RuntimeError(f"axon_stop_nrt_profile rc={n}")

    mod = types.ModuleType("antenv.axon_hooks")
    mod.get_axon_ntff_profile_hook = lambda: hook
    mod.set_axon_ntff_profile_hook = lambda h: None
    sys.modules["antenv.axon_hooks"] = mod


def kernel(x_dftb, coords, dst_idx, src_idx, W0, b0, Wmp, bmp, W1, b1, W2, b2):

    x_dftb = np.asarray(x_dftb, np.float32)
    coords = np.asarray(coords, np.float32)
    dst_idx = np.asarray(dst_idx).astype(np.int64)
    src_idx = np.asarray(src_idx).astype(np.int64)

    cores, cap, ntiles, cht = prep_cores(coords, dst_idx, src_idx)
    layf, totf = blob_layout(cap, ntiles)

    M0 = build_embed_mat(np.asarray(W0), np.asarray(b0), FIN, F)
    M1 = build_embed_mat(np.asarray(W1), np.asarray(b1), F, F)
    M2 = build_embed_mat(np.asarray(W2), np.asarray(b2), F, 1)
    M0a_h = M0[0:P]
    # bias row first, then the 16 remaining weight rows (x2c row 0 is the
    # device-memset ones row)
    M0b_h = np.concatenate([M0[144:145], M0[P:144]], 0)

    def kchunks(M, ncols):
        out = np.zeros((3, 97, ncols), np.float32)
        for kk in range(3):
            out[kk, 0:96] = M[kk * 96:(kk + 1) * 96]
        out[2, 96] = M[288]
        return out

    M1e_h = kchunks(M1, 288).transpose(1, 0, 2).reshape(97, 3 * 288)
    M2e_h = kchunks(M2, 9).transpose(1, 0, 2).reshape(97, 3 * 9)
    Wstk_h = np.concatenate(
        [build_wstack(np.asarray(Wmp)[s], np.asarray(bmp)[s])
         for s in range(2)], 1)  # [33, 2*NTRI*F]
    Gexp_h = build_gexp()

    nc = build_program(cap, ntiles, cht)
    nc.finalize()

    wlay, wtot = weights_layout()
    wfull_h = np.zeros(wtot, np.float32)
    for name, arr in (("M0a", M0a_h), ("M0b", M0b_h), ("Wstk", Wstk_h),
                      ("Gexp", Gexp_h),
                      ("IotaF", np.broadcast_to(
                          np.arange(P, dtype=np.float32), (P, P))),
                      ("M1e", M1e_h), ("M2e", M2e_h)):
        off, rows, cols = wlay[name]
        wfull_h[off:off + rows * cols] = np.asarray(
            arr, np.float32).reshape(-1)

    xd = x_dftb.reshape(E, 144)
    in_maps = []
    for k in range(NCORES):
        c = cores[k]
        blk = xd[k * NB:(k + 1) * NB]
        bf = np.zeros(totf, np.float32)
        def putf(name, arr):
            off, rows, cols = layf[name]
            bf[off:off + rows * cols] = np.asarray(
                arr, np.float32).reshape(-1)
        putf("xdT1", blk[:, 0:P].T)
        putf("xdT2", blk[:, P:144].T)
        putf("radT", c['radT'][:B])
        putf("YshT", c['YshT'][1:9])
        putf("colidx", c['colidx'].T)
        putf("wts", wfull_h)
        bi = np.concatenate([c['srclo'].T, c['srchi'].T, c['sidx'].T], 1)
        in_maps.append({
            "blobf": bf,
            "blobi": np.ascontiguousarray(bi, np.int32),
        })

    import time
    from concourse import bass_utils

    _install_axon_profile_hooks()
    t0 = time.time()
    br = bass_utils.run_bass_kernel_spmd(
        nc, in_maps, core_ids=list(range(NCORES)), trace=True,
        trace_cores=list(range(NCORES)))
    wall_ns = int((time.time() - t0) * 1e9)
    global LAST_EXEC_NS, LAST_TRACE
    # exec_time_ns is the neuron-profiled NEFF execution time on device,
    # max over the 8 cores. Fall back to call wall time (which includes
    # tunnel transfer + jit compile) only if profiling was unavailable.
    LAST_EXEC_NS = br.exec_time_ns if br.exec_time_ns else wall_ns
    LAST_TRACE = br.instructions_and_trace[1] \
        if br.instructions_and_trace else None
    out = np.zeros((E, 1, 9, 1), np.float32)
    for k in range(NCORES):
        out[k * NB:(k + 1) * NB, 0, :, 0] = \
            br.results[k]["outG"].astype(np.float32).T  # [9, NB] f16
    return out



# revision 18
# speedup vs baseline: 1.0764x; 1.0124x over previous
"""Trainium2 Bass kernel for nn_CoeffNet (equivariant GNN message passing).

Sharding: edges bucketed by destination-node block (8 blocks of 16384 nodes,
one per core) and sorted by dst; scatter is purely local per core; updated
node-feature blocks are exchanged between the two message-passing steps with
an on-chip AllGather. Small weights travel as 1/8 slices per core and are
reassembled on-chip by a 674 KB AllGather; constant rows (embed bias, rad
ones, Ysh l=0) are memset on device instead of transferred.

Inputs are packed into two consolidated per-core blobs (one f32, one
int32). The dedupe 0/1 matrix S (8.5 MB/core as f32 in the original
formulation) is reconstructed on device from a 66 KB column-index vector
via an iota/is_equal compare. The value path stays f32 end to end: the
equivariant gate (sign of the scalar channel) has gate margins down to
~1e-7 on this input, so quantizing x/rad/Ysh to fp16 or even 24-bit fixed
point flips gates and breaks correctness (verified by host simulation).
Only the final output is fp16 (no feedback).

Execution + measurement: the kernel runs via bass_utils.run_bass_kernel_spmd
on cores 0-7 with trace=True, which wraps the NEFF execution in NRT/NTFF
profiling (neuron-profile) and returns the hardware execution time of the
NEFF as measured on device, max over the 8 cores. LAST_EXEC_NS is that
neuron-profiled device execution time. (The agent image lacks
antenv.axon_hooks, so an equivalent ctypes NTFF hook against
libaxon_pjrt.so is installed first; the artifact-bucket upload is stubbed
to a local path since this container has no artifact store.)

Per core, one NEFF:
  embed (PE, per-tile stationary xdT chunks)  -> x0 block -> AllGather
  2x message-pass step:
      per 128-edge tile: indirect-DMA gather of xs rows from the gathered
      full feature table; coef = radT.T @ Wstack (PE); u' = YshT.T @ Gexp
      (PE); tensor product on DVE (triple-grouped wide ops); in-tile dedupe
      matmul (St one-hot built on DVE from colidx); indirect scatter-add
      into the residual staging block (rows globally unique by host
      construction).
      AllGather staging -> full table (before each step).
  final: per-tile PE transposes, W1 (PE), equivariant gate, W2 (PE) -> outT
  (fp16).
Host does: index bucketing/sorting/padding, Ysh/rad basis evaluation,
weight reshaping, blob packing, output concat+transpose.
"""
import numpy as np
from contextlib import ExitStack

E = 131072
FIN = 16
F = 32
B = 32
NCORES = 8
NB = E // NCORES
P = 128
SLC = [(0, 1), (1, 4), (4, 9)]
CENTERS = np.linspace(0.0, 4.0, B).astype(np.float32)
COLPAD = 500.0  # colidx sentinel for invalid edges (exact in fp16)

C0 = 0.28209479177387814
C1 = 0.4886025119029199
C2 = 1.0925484305920792
C20 = 0.31539156525252005
C22 = 0.5462742152960396


def _sh9_np(u):
    x, y, z = u[..., 0], u[..., 1], u[..., 2]
    return np.stack([
        np.full_like(x, C0), C1 * y, C1 * z, C1 * x,
        C2 * x * y, C2 * y * z, C20 * (3.0 * z * z - 1.0), C2 * x * z,
        C22 * (x * x - y * y)
    ], axis=-1)


def _gaunt_np():
    ct, w = np.polynomial.legendre.leggauss(8)
    phi = np.arange(16) * (2.0 * np.pi / 16.0)
    st = np.sqrt(1.0 - ct**2)
    X = (st[:, None] * np.cos(phi)[None, :]).ravel()
    Yc = (st[:, None] * np.sin(phi)[None, :]).ravel()
    Z = np.repeat(ct, 16)
    wq = np.repeat(w, 16) * (2.0 * np.pi / 16.0)
    Yg = _sh9_np(np.stack([X, Yc, Z], axis=-1))
    return np.einsum('qa,qb,qc,q->abc', Yg, Yg, Yg, wq)


GAUNT = _gaunt_np().astype(np.float32)
ALLOWED = [(l1, l2, l3)
           for l1 in range(3) for l2 in range(3) for l3 in range(3)
           if np.abs(GAUNT[SLC[l1][0]:SLC[l1][1], SLC[l2][0]:SLC[l2][1],
                          SLC[l3][0]:SLC[l3][1]]).max() > 1e-8]
NTRI = len(ALLOWED)  # 11

# Packed nonzero-term layout: per (t, c) only the a's with a nonzero Gaunt
# column survive, stored as strided runs (start, stride, count) so pi-mult
# APs stay regular. 115 dense terms -> 79 nonzero.
def _build_tc_info():
    info = []  # (t_idx, c_local, runs=[(s, d, n, gcol_off)])
    off = 0
    for t_idx, (l1, l2, l3) in enumerate(ALLOWED):
        (a0, a1), (b0, b1), (c0, c1) = SLC[l1], SLC[l2], SLC[l3]
        for c in range(c0, c1):
            nz = [a - a0 for a in range(a0, a1)
                  if np.abs(GAUNT[a, b0:b1, c]).max() > 1e-8]
            assert nz, (t_idx, c)
            runs = []
            i = 0
            while i < len(nz):
                if i + 1 < len(nz):
                    d = nz[i + 1] - nz[i]
                    j = i + 1
                    while j + 1 < len(nz) and nz[j + 1] - nz[j] == d:
                        j += 1
                else:
                    d, j = 1, i
                runs.append((nz[i], max(d, 1), j - i + 1, off))
                off += j - i + 1
                i = j + 1
            info.append((t_idx, c - c0, runs))
    return info, off


TC_INFO, NTERM = _build_tc_info()  # NTERM = 79 packed columns
NUP = -(-NTERM // 4) * 4


def build_gexp():
    M = np.zeros((9, NUP), np.float32)
    for t_idx, c_local, runs in TC_INFO:
        l1, l2, l3 = ALLOWED[t_idx]
        (a0, a1), (b0, b1), (c0, c1) = SLC[l1], SLC[l2], SLC[l3]
        c = c0 + c_local
        for (s, d, n, off) in runs:
            for ei in range(n):
                a = a0 + s + ei * d
                M[b0:b1, off + ei] = GAUNT[a, b0:b1, c]
    return M


def build_embed_mat(W, b, fin, fout):
    M = np.zeros((9 * fin + 1, 9 * fout), np.float32)
    for l, (s0, s1) in enumerate(SLC):
        for c in range(s0, s1):
            M[c * fin:(c + 1) * fin, c * fout:(c + 1) * fout] = W[l]
    M[9 * fin, 0:fout] = np.asarray(b).reshape(-1)
    return M


def build_wstack(Wmp_s, bmp_s):
    M = np.zeros((B + 1, NTRI * F), np.float32)
    for t_idx, (l1, l2, l3) in enumerate(ALLOWED):
        M[:B, t_idx * F:(t_idx + 1) * F] = Wmp_s[l1, l2, l3]
        M[B, t_idx * F:(t_idx + 1) * F] = bmp_s[l1, l2, l3]
    return M


def prep_cores(coords, dst_idx, src_idx):
    """Slab-aligned edge layout: per core, edges are bucketed by dst slab
    (8 slabs of 2048 nodes), dst-sorted and group-padded within each slab,
    and every slab is padded to a common tile count Ts so tile->slab is
    compile-time (tile gt belongs to slab gt // Ts on every core)."""
    rel = coords[dst_idx] - coords[src_idx]
    r = np.sqrt(np.sum(rel * rel, axis=-1) + 1e-8)
    u = rel / r[:, None]
    Ysh = _sh9_np(u).astype(np.float32)
    rad = np.exp(-(r[:, None] - CENTERS[None, :])**2).astype(np.float32)
    rad1 = np.concatenate([rad, np.ones((E, 1), np.float32)], 1)
    NSLAB = NB // 2048
    HROWS = NCORES * (NB // 2)  # rows per half-table

    def build_stream(eids, dl):
        out_src, out_dst, out_eid = [], [], []
        cur, i, n = 0, 0, len(eids)
        while i < n:
            j = i
            while j < n and dl[j] == dl[i]:
                j += 1
            glen = j - i
            assert glen <= P, "dst in-degree exceeds one tile"
            room = (-cur) % P
            if room != 0 and glen > room:
                out_src += [0] * room
                out_dst += [-1] * room
                out_eid += [-1] * room
                cur += room
            out_src += [int(v) for v in src_idx[eids[i:j]]]
            out_dst += [int(dl[i])] * glen
            out_eid += [int(v) for v in eids[i:j]]
            cur += glen
            i = j
        return out_src, out_dst, out_eid

    streams = []
    for k in range(NCORES):
        e_ids = np.nonzero((dst_idx // NB) == k)[0]
        order = np.argsort(dst_idx[e_ids], kind='stable')
        e_ids = e_ids[order]
        dloc = dst_idx[e_ids] - k * NB
        row = []
        for s in range(NSLAB):
            m = (dloc // 2048) == s
            row.append(build_stream(e_ids[m], dloc[m] - s * 2048))
        streams.append(row)

    maxlen = max(len(st[0]) for row in streams for st in row)
    Ts = -(-maxlen // P)
    if Ts % 2:
        Ts += 1
    ntiles = NSLAB * Ts
    cap = ntiles * P
    cht = Ts // 2

    cores = []
    for k in range(NCORES):
        srca = np.zeros(cap, np.int64)
        dsta = np.full(cap, -1, np.int64)   # slab-local dst [0, 2048)
        eida = np.full(cap, -1, np.int64)
        for s in range(NSLAB):
            osrc, odst, oeid = streams[k][s]
            o0 = s * Ts * P
            srca[o0:o0 + len(osrc)] = osrc
            dsta[o0:o0 + len(osrc)] = odst
            eida[o0:o0 + len(osrc)] = oeid
        c = dict(src=srca, dst=dsta, eid=eida)
        valid = eida >= 0
        YshT = np.zeros((9, cap), np.float32)
        radT = np.zeros((B + 1, cap), np.float32)
        YshT[:, valid] = Ysh[eida[valid]].T
        radT[:, valid] = rad1[eida[valid]].T
        c['YshT'] = YshT
        c['radT'] = radT
        # gather indices split into lo/hi half-tables (row = owner*8192 +
        # local_row_within_half); out-of-half entries are OOB-skipped
        g = srca
        j = g // NB
        rl = g % NB
        OOB = 1 << 20
        lo = np.where(rl < NB // 2, j * (NB // 2) + rl, OOB)
        hi = np.where(rl >= NB // 2, j * (NB // 2) + (rl - NB // 2), OOB)
        c['srclo'] = lo.astype(np.int32).reshape(ntiles, P)
        c['srchi'] = hi.astype(np.int32).reshape(ntiles, P)
        # scatter targets the half-staging tensor of the tile's slab;
        # sidx is half-local ((s%4)*2048 + slab-local dst), trash = NB//2
        colidx = np.full((ntiles, P), COLPAD, np.float32)
        sidx = np.full((ntiles, P), NB // 2, np.int32)
        d = c['dst']
        for t in range(ntiles):
            s = t // Ts
            dt = d[t * P:(t + 1) * P]
            first = {}
            for e2 in range(P):
                if dt[e2] < 0:
                    continue
                if dt[e2] not in first:
                    first[dt[e2]] = e2
                    sidx[t, e2] = (s % 4) * 2048 + dt[e2]
                colidx[t, e2] = first[dt[e2]]
        c['colidx'] = colidx
        c['sidx'] = sidx
        cores.append(c)
    return cores, cap, ntiles, cht


def weights_layout():
    """(name -> (offset, rows, cols)) for the weights region of the per-core
    blob (full weights on every core; no on-chip reassembly)."""
    lay = {}
    off = 0
    for name, rows, cols in (
            ("M0a", P, 288), ("M0b", 17, 288),
            ("Wstk", B + 1, 2 * NTRI * F), ("Gexp", 9, NUP),
            ("IotaF", P, P), ("M1e", 97, 3 * 288), ("M2e", 97, 3 * 9)):
        lay[name] = (off, rows, cols)
        off += rows * cols
    return lay, off


def blob_layout(cap, ntiles):
    """(name -> (offset, rows, cols)) for the per-core f32 blob.  Constant
    rows (xdT2 bias row, rad ones row, Ysh l=0 row) are memset on device."""
    _, wtot = weights_layout()
    lay = {}
    off = 0
    for name, rows, cols in (
            ("xdT1", P, NB), ("xdT2", 16, NB),
            ("radT", B, cap), ("YshT", 8, cap),
            ("colidx", P, ntiles), ("wts", 1, wtot)):
        lay[name] = (off, rows, cols)
        off += rows * cols
    return lay, off


def build_program(cap, ntiles, cht):
    import concourse.bass as bass
    import concourse.bacc as bacc
    import concourse.mybir as mybir
    import concourse.tile as tile
    from concourse.masks import make_identity
    f32 = mybir.dt.float32
    f16 = mybir.dt.float16
    i32 = mybir.dt.int32
    AOP = mybir.AluOpType
    AP = bass.AP

    T = cht
    nchunks = ntiles // T
    layf, totf = blob_layout(cap, ntiles)
    wlay, wtot = weights_layout()

    nc = bacc.Bacc("TRN2", target_bir_lowering=False, debug=False,
                   num_devices=NCORES)

    blobf = nc.dram_tensor("blobf", [totf], f32, kind="ExternalInput").ap()
    blobi = nc.dram_tensor("blobi", [P, 3 * ntiles], i32,
                           kind="ExternalInput").ap()
    outG = nc.dram_tensor("outG", [9, NB], f16, kind="ExternalOutput").ap()

    def vf(name, coloff=0, ncols=None):
        off, rows, cols = layf[name]
        if ncols is None:
            ncols = cols - coloff
        return AP(blobf.tensor, off + coloff, [[cols, rows], [1, ncols]])

    def bc(ap, lvl, n):
        """insert a [0, n] broadcast level at free position lvl (0-based
        after partition dim)"""
        raw = list(ap.ap)
        raw.insert(1 + lvl, [0, n])
        return AP(ap.tensor, ap.offset, raw)

    NEMB = NB // 2048          # embed chunks == slabs
    Ts = ntiles // NEMB        # tiles per slab (T = Ts // 2)
    HROWS = NCORES * (NB // 2)  # rows per half gather-table
    HBC = HROWS - 1            # gather bounds check (greater => skipped)

    with tile.TileContext(nc) as tc, ExitStack() as ctx:
        dram = ctx.enter_context(tc.tile_pool(name="dram", bufs=1,
                                              space="DRAM"))
        const = ctx.enter_context(tc.tile_pool(name="const", bufs=1))

        # two half-staging tensors (half h = node rows [h*8192,(h+1)*8192)
        # + its own trash row 8192): scatters target the compile-time-known
        # half, so an AllGather of one half never false-serializes against
        # scatters into the other half
        stgh = [dram.tile([NB // 2 + 1, 288], f32, name=f"stg{h}")
                for h in range(2)]
        # two Shared half-tables per step (Shared output allows a single
        # writer only, and Shared-output collectives are ~2x faster than
        # Local-output ones); gathers do a lo+hi pair with OOB skip
        xf = [[dram.tile([HROWS, 288], f32, addr_space="Shared",
                         name=f"xf{st}{h}") for h in range(2)]
              for st in range(2)]
        myoutT = dram.tile([9, NB], f16)

        wts = vf("wts")

        def wview(name):
            off, rows, cols = wlay[name]
            return AP(wts.tensor, wts.offset + off,
                      [[cols, rows], [1, cols]])

        M0a_s = const.tile([P, 288], f32)
        nc.sync.dma_start(out=M0a_s[:], in_=wview("M0a"))
        M0b_s = const.tile([17, 288], f32)
        nc.sync.dma_start(out=M0b_s[:], in_=wview("M0b"))
        Wstk_s = const.tile([B + 1, 2 * NTRI * F], f32)
        nc.sync.dma_start(out=Wstk_s[:], in_=wview("Wstk"))
        Gexp_s = const.tile([9, NUP], f32)
        nc.sync.dma_start(out=Gexp_s[:], in_=wview("Gexp"))
        M1e_s = const.tile([97, 3 * 288], f32)
        nc.sync.dma_start(out=M1e_s[:], in_=wview("M1e"))
        M2e_s = const.tile([97, 3 * 9], f32)
        nc.sync.dma_start(out=M2e_s[:], in_=wview("M2e"))
        IotaF = const.tile([P, P], f32)
        nc.sync.dma_start(out=IotaF[:], in_=wview("IotaF"))
        ident = const.tile([P, P], f32)
        make_identity(nc, ident[:])
        srclo_s = const.tile([P, ntiles], i32)
        nc.sync.dma_start(out=srclo_s[:], in_=blobi[:, 0:ntiles])
        srchi_s = const.tile([P, ntiles], i32)
        nc.sync.dma_start(out=srchi_s[:], in_=blobi[:, ntiles:2 * ntiles])
        sidx_s = const.tile([P, ntiles], i32)
        nc.sync.dma_start(out=sidx_s[:], in_=blobi[:, 2 * ntiles:3 * ntiles])
        colidx_f = const.tile([P, ntiles], f32)
        nc.sync.dma_start(out=colidx_f[:], in_=vf("colidx"))

        def half_ag(step, h):
            nc.gpsimd.collective_compute(
                "AllGather", AOP.bypass,
                replica_groups=[list(range(NCORES))],
                ins=[stgh[h][0:NB // 2, :].opt()],
                outs=[xf[step][h][:].opt()])

        # ---- embed (half AllGathers overlap the remaining embed) ----
        x2cs = [const.tile([17, 2048], f32, name=f"x2c{i}") for i in range(2)]
        for t in x2cs:
            nc.vector.memset(t[0:1, :], 1.0)
        with tc.spectator_scope("embed"), \
             tc.tile_pool(name="emb", bufs=2) as emb, \
             tc.tile_pool(name="emb3", bufs=3) as emb3, \
             tc.tile_pool(name="psE", bufs=2, space="PSUM") as ps:
            for ch in range(NEMB):
                x1c = emb.tile([P, 2048], f32, tag="x1c")
                nc.sync.dma_start(out=x1c[:],
                                  in_=vf("xdT1", ch * 2048, 2048))
                x2c = x2cs[ch % 2]
                nc.sync.dma_start(out=x2c[1:17, :],
                                  in_=vf("xdT2", ch * 2048, 2048))
                for ti in range(16):
                    x0p = ps.tile([P, 288], f32, tag="x0p")
                    sl = slice(ti * P, (ti + 1) * P)
                    nc.tensor.matmul(out=x0p[:], lhsT=x1c[:, sl],
                                     rhs=M0a_s[:], start=True, stop=False)
                    nc.tensor.matmul(out=x0p[:], lhsT=x2c[:, sl],
                                     rhs=M0b_s[:], start=False, stop=True)
                    x0s = emb3.tile([P, 288], f32, tag="x0s")
                    nc.scalar.copy(out=x0s[:], in_=x0p[:])
                    node0 = ch * 2048 + ti * P
                    hh, loc = node0 // (NB // 2), node0 % (NB // 2)
                    nc.sync.dma_start(out=stgh[hh][loc:loc + P, :],
                                      in_=x0s[:])
                if ch == NEMB // 2 - 1:
                    half_ag(0, 0)
                elif ch == NEMB - 1:
                    half_ag(0, 1)

        # ---- message passing + final (final interleaved into step 1) ----
        radcs = [const.tile([B + 1, T * P], f32, name=f"radc{i}")
                 for i in range(2)]
        yshcs = [const.tile([9, T * P], f32, name=f"yshc{i}")
                 for i in range(2)]
        for t in radcs:
            nc.vector.memset(t[B:B + 1, :], 1.0)
        for t in yshcs:
            nc.vector.memset(t[0:1, :], C0)
        with tc.tile_pool(name="mp", bufs=2) as mp, \
             tc.tile_pool(name="mp3", bufs=3) as mp3, \
             tc.tile_pool(name="fin", bufs=1) as fin, \
             tc.tile_pool(name="fin2", bufs=2) as fin2, \
             tc.tile_pool(name="psM", bufs=1, space="PSUM") as ps, \
             tc.tile_pool(name="psC", bufs=2, space="PSUM") as psc, \
             tc.tile_pool(name="psD", bufs=1, space="PSUM") as psd, \
             tc.tile_pool(name="psF", bufs=1, space="PSUM") as psf:

            x2T_t = const.tile([97, 3, 8, P], f32, name="x2Tt")
            nc.vector.memset(x2T_t[96:97, :, :, :], 1.0)
            y1g_t = const.tile([97, 3, 8, P], f32, name="y1gt")
            nc.vector.memset(y1g_t[96:97, :, :, :], 1.0)

            def emit_final(f):
                """W1 -> gate -> W2 for node rows [f*1024, (f+1)*1024)."""
                n0 = f * 1024
                hh, l0 = n0 // (NB // 2), n0 % (NB // 2)
                xtb = fin.tile([P, 8, 288], f32, tag="xtb")
                for ti in range(8):
                    nc.sync.dma_start(
                        out=xtb[:, ti, :],
                        in_=stgh[hh][l0 + ti * P:l0 + (ti + 1) * P, :])
                x2T = x2T_t
                for kk in range(3):
                    for tb in range(2):
                        tp = psf.tile([96, 4, P], f32, tag="tp",
                                      name=f"tp{kk}{tb}")
                        for t4 in range(4):
                            ti = tb * 4 + t4
                            nc.tensor.transpose(
                                out=tp[:, t4, :],
                                in_=xtb[:, ti, kk * 96:(kk + 1) * 96],
                                identity=ident[:])
                        nc.scalar.copy(
                            out=x2T[0:96, kk, tb * 4:(tb + 1) * 4, :],
                            in_=tp[:])
                y1a = fin.tile([96, 3, 8, P], f32, tag="y1a")
                for m in range(3):
                    for tb in range(2):
                        y1p = psf.tile([96, 4, P], f32, tag="y1p")
                        for kk in range(3):
                            nc.tensor.matmul(
                                out=y1p[:],
                                lhsT=M1e_s[:, m * 288 + kk * 96:
                                           m * 288 + (kk + 1) * 96],
                                rhs=x2T[:, kk, tb * 4:(tb + 1) * 4, :],
                                start=(kk == 0), stop=(kk == 2))
                        nc.scalar.copy(
                            out=y1a[:, m, tb * 4:(tb + 1) * 4, :],
                            in_=y1p[:])
                maskx = fin2.tile([96, 8, P], f32, tag="maskx")
                for cl in range(3):
                    nc.vector.tensor_scalar(
                        out=maskx[cl * F:(cl + 1) * F, :, :],
                        in0=y1a[0:F, 0, :, :],
                        scalar1=0.0, scalar2=None, op0=AOP.is_gt)
                y1g = y1g_t
                for m in range(3):
                    nc.vector.tensor_tensor(
                        out=y1g[0:96, m, :, :], in0=y1a[:, m, :, :],
                        in1=maskx[:], op=AOP.mult)
                y2s = fin2.tile([9, 8, P], f16, tag="y2s")
                for tb in range(2):
                    y2p = psf.tile([9, 4, P], f32, tag="tp",
                                   name=f"y2p{tb}")
                    for m in range(3):
                        nc.tensor.matmul(
                            out=y2p[:],
                            lhsT=M2e_s[:, m * 9:(m + 1) * 9],
                            rhs=y1g[:, m, tb * 4:(tb + 1) * 4, :],
                            start=(m == 0), stop=(m == 2))
                    nc.scalar.copy(out=y2s[:, tb * 4:(tb + 1) * 4, :],
                                   in_=y2p[:])
                nc.sync.dma_start(
                    out=myoutT[:, n0:n0 + 1024]
                        .rearrange("g (t p) -> g t p", p=P),
                    in_=y2s[:])

            for step in range(2):
                lo_t, hi_t = xf[step]
                wof = step * NTRI * F
                mpscope = tc.spectator_scope(f"mp{step}")
                mpscope.__enter__()
                for chn in range(nchunks):
                    t0 = chn * T
                    xs = mp.tile([P, T, 288], f32, tag="xs")
                    coef = mp.tile([P, T, NTRI * F], f32, tag="coef")
                    up = mp.tile([P, T, NUP], f32, tag="up")
                    msg = mp.tile([P, T, 288], f32, tag="msg")
                    radc = radcs[chn % 2]
                    nc.sync.dma_start(out=radc[0:B, :],
                                      in_=vf("radT", t0 * P, T * P))
                    yshc = yshcs[chn % 2]
                    nc.sync.dma_start(out=yshc[1:9, :],
                                      in_=vf("YshT", t0 * P, T * P))
                    # all lo-gathers first: they only depend on the lo
                    # half-table, so they can run while the hi AllGather is
                    # still in flight at a step boundary
                    for ti in range(T):
                        gt = t0 + ti
                        nc.gpsimd.indirect_dma_start(
                            out=xs[:, ti, :], out_offset=None,
                            in_=lo_t[:],
                            in_offset=bass.IndirectOffsetOnAxis(
                                ap=srclo_s[:, gt:gt + 1], axis=0),
                            bounds_check=HBC, oob_is_err=False)
                    for ti in range(T):
                        gt = t0 + ti
                        nc.gpsimd.indirect_dma_start(
                            out=xs[:, ti, :], out_offset=None,
                            in_=hi_t[:],
                            in_offset=bass.IndirectOffsetOnAxis(
                                ap=srchi_s[:, gt:gt + 1], axis=0),
                            bounds_check=HBC, oob_is_err=False)
                    for ti in range(T):
                        gt = t0 + ti
                        cu = psc.tile([P, NTRI * F + NUP], f32, tag="cu")
                        nc.tensor.matmul(
                            out=cu[:, 0:NTRI * F],
                            lhsT=radc[:, ti * P:(ti + 1) * P],
                            rhs=Wstk_s[:, wof:wof + NTRI * F],
                            start=True, stop=True)
                        nc.scalar.copy(out=coef[:, ti, :],
                                       in_=cu[:, 0:NTRI * F])
                        nc.tensor.matmul(
                            out=cu[:, NTRI * F:NTRI * F + NUP],
                            lhsT=yshc[:, ti * P:(ti + 1) * P],
                            rhs=Gexp_s[:], start=True, stop=True)
                        nc.scalar.copy(out=up[:, ti, :],
                                       in_=cu[:, NTRI * F:NTRI * F + NUP])
                    # tensor product (Gaunt-sparse, strided runs); c-degree
                    # accumulation on TensorE (identity matmuls into PSUM),
                    # three 3-c waves sharing one 3-bank PSUM slot
                    for wave, clo, chi in ((0, 0, 3), (1, 3, 6), (2, 6, 9)):
                      NC_W = chi - clo
                      wave_info = [
                          (t_idx, c_local, runs)
                          for t_idx, c_local, runs in TC_INFO
                          if clo <= SLC[ALLOWED[t_idx][2]][0] + c_local < chi]
                      last_op = {}
                      for t_idx, c_local, runs in wave_info:
                        cgl = SLC[ALLOWED[t_idx][2]][0] + c_local
                        for ri, (s, d, n, off) in enumerate(runs):
                            for ai in range(n):
                                last_op[cgl] = (t_idx, ri, ai)
                      msgp = ps.tile([P, NC_W, T * F], f32, tag="msgp",
                                     padded_shape=[P, NC_W, 512],
                                     name=f"msgp{wave}")
                      first_c = [True] * 9
                      qcache = {}
                      for t_idx, c_local, runs in wave_info:
                        l1, l2, l3 = ALLOWED[t_idx]
                        (a0, a1), (c0, c1) = SLC[l1], SLC[l3]
                        na = a1 - a0
                        if t_idx not in qcache:
                            q = mp.tile([P, T, 160], f32, tag="q",
                                        name=f"q{t_idx}")
                            nc.vector.tensor_tensor(
                                out=q[:, :, 0:na * F]
                                    .rearrange("p t (a f) -> p t a f", f=F),
                                in0=xs[:, :, a0 * F:a1 * F]
                                    .rearrange("p t (a f) -> p t a f", f=F),
                                in1=bc(coef[:, :, t_idx * F:(t_idx + 1) * F],
                                       1, na),
                                op=AOP.mult)
                            qcache[t_idx] = q
                        q = qcache[t_idx]
                        cgl = c0 + c_local
                        for ri, (s, d, n, off) in enumerate(runs):
                            pi = mp.tile([P, T, 160], f32, tag="pi")
                            qb = q[:, :, 0:160]
                            qrun = AP(qb.tensor, qb.offset + s * F,
                                      [qb.ap[0], qb.ap[1],
                                       [d * F, n], [1, F]])
                            nc.vector.tensor_tensor(
                                out=pi[:, :, 0:n * F]
                                    .rearrange("p t (a f) -> p t a f", f=F),
                                in0=qrun,
                                in1=bc(up[:, :, off:off + n], 2, F),
                                op=AOP.mult)
                            for ai in range(n):
                                nc.tensor.matmul(
                                    out=msgp[:, cgl - clo, :]
                                        .rearrange("p (t f) -> p t f",
                                                   f=F),
                                    lhsT=ident[:],
                                    rhs=pi[:, :, ai * F:(ai + 1) * F],
                                    start=first_c[cgl],
                                    stop=(last_op[cgl]
                                          == (t_idx, ri, ai)))
                                first_c[cgl] = False
                      for cgl in range(clo, chi):
                        nc.scalar.copy(
                            out=msg[:, :, cgl * F:(cgl + 1) * F],
                            in_=msgp[:, cgl - clo, :]
                                .rearrange("p (t f) -> p t f", f=F))
                    # dedupe + scatter (out view limited to the tile's slab
                    # tail so range-based deps stay slab-granular)
                    for ti in range(T):
                        gt = t0 + ti
                        slab = gt // Ts
                        St = mp3.tile([P, P], f32, tag="St")
                        nc.vector.tensor_tensor(
                            out=St[:], in0=IotaF[:],
                            in1=bc(colidx_f[:, gt:gt + 1], 0, P),
                            op=AOP.is_equal)
                        ddp = psd.tile([P, 288], f32, tag="ddp")
                        nc.tensor.matmul(out=ddp[:], lhsT=St[:],
                                         rhs=msg[:, ti, :],
                                         start=True, stop=True)
                        scs = mp3.tile([P, 288], f32, tag="scs")
                        nc.scalar.copy(out=scs[:], in_=ddp[:])
                        nc.gpsimd.indirect_dma_start(
                            out=stgh[slab // 4][:],
                            out_offset=bass.IndirectOffsetOnAxis(
                                ap=sidx_s[:, gt:gt + 1], axis=0),
                            in_=scs[:], in_offset=None,
                            compute_op=AOP.add)
                    if step == 0:
                        if chn == nchunks // 2 - 1:
                            half_ag(1, 0)
                        elif chn == nchunks - 1:
                            half_ag(1, 1)
                    else:
                        if chn % 2 == 1:
                            emit_final(chn - 1)
                            emit_final(chn)
                mpscope.__exit__(None, None, None)

        nc.sync.dma_start(out=outG[:], in_=myoutT[:])

    return nc


LAST_EXEC_NS = None
_AXON_SO_PATH = "/opt/axon/libaxon_pjrt.so"


def _install_axon_profile_hooks():
    """Make bass_utils' axon NTFF-profiling path work on this image.

    run_bass_kernel_spmd(trace=True) under axon needs
    antenv.axon_hooks.get_axon_ntff_profile_hook(); the agent image ships
    without that module, so synthesize it with the slim ctypes hook
    (identical to trn_agent_boot.trn_boot._ntff_profile_via_ctypes). Also
    stub the artifact-bucket upload: no artifact store in this container.
    """
    import contextlib
    import ctypes
    import sys
    import types
    from concourse import bass_utils

    bass_utils.upload_artifacts = lambda tmpdir, **kw: "local://" + str(tmpdir)
    try:
        from antenv.axon_hooks import get_axon_ntff_profile_hook  # noqa: F401
        return
    except ImportError:
        pass

    lib = ctypes.CDLL(_AXON_SO_PATH)
    if not hasattr(lib, "axon_start_nrt_profile"):
        hook = None
    else:
        lib.axon_start_nrt_profile.argtypes = [
            ctypes.POINTER(ctypes.c_int64), ctypes.c_size_t]
        lib.axon_start_nrt_profile.restype = ctypes.c_int64
        lib.axon_stop_nrt_profile.argtypes = [ctypes.c_char_p]
        lib.axon_stop_nrt_profile.restype = ctypes.c_int64

        @contextlib.contextmanager
        def hook(output_dir, device_ids):
            import jax
            jax.devices()
            if device_ids:
                ids = (ctypes.c_int64 * len(device_ids))(*device_ids)
                rc = lib.axon_start_nrt_profile(ids, len(device_ids))
            else:
                rc = lib.axon_start_nrt_profile(None, 0)
            if rc != 0:
                raise RuntimeError(f"axon_start_nrt_profile rc={rc}")
            try:
                yield
            finally:
                n = lib.axon_stop_nrt_profile(str(output_dir).encode())
                if n < 0:
                    raise RuntimeError(f"axon_stop_nrt_profile rc={n}")

    mod = types.ModuleType("antenv.axon_hooks")
    mod.get_axon_ntff_profile_hook = lambda: hook
    mod.set_axon_ntff_profile_hook = lambda h: None
    sys.modules["antenv.axon_hooks"] = mod


def kernel(x_dftb, coords, dst_idx, src_idx, W0, b0, Wmp, bmp, W1, b1, W2, b2):

    x_dftb = np.asarray(x_dftb, np.float32)
    coords = np.asarray(coords, np.float32)
    dst_idx = np.asarray(dst_idx).astype(np.int64)
    src_idx = np.asarray(src_idx).astype(np.int64)

    cores, cap, ntiles, cht = prep_cores(coords, dst_idx, src_idx)
    layf, totf = blob_layout(cap, ntiles)

    M0 = build_embed_mat(np.asarray(W0), np.asarray(b0), FIN, F)
    M1 = build_embed_mat(np.asarray(W1), np.asarray(b1), F, F)
    M2 = build_embed_mat(np.asarray(W2), np.asarray(b2), F, 1)
    M0a_h = M0[0:P]
    # bias row first, then the 16 remaining weight rows (x2c row 0 is the
    # device-memset ones row)
    M0b_h = np.concatenate([M0[144:145], M0[P:144]], 0)

    def kchunks(M, ncols):
        out = np.zeros((3, 97, ncols), np.float32)
        for kk in range(3):
            out[kk, 0:96] = M[kk * 96:(kk + 1) * 96]
        out[2, 96] = M[288]
        return out

    M1e_h = kchunks(M1, 288).transpose(1, 0, 2).reshape(97, 3 * 288)
    M2e_h = kchunks(M2, 9).transpose(1, 0, 2).reshape(97, 3 * 9)
    Wstk_h = np.concatenate(
        [build_wstack(np.asarray(Wmp)[s], np.asarray(bmp)[s])
         for s in range(2)], 1)  # [33, 2*NTRI*F]
    Gexp_h = build_gexp()

    nc = build_program(cap, ntiles, cht)
    nc.finalize()

    wlay, wtot = weights_layout()
    wfull_h = np.zeros(wtot, np.float32)
    for name, arr in (("M0a", M0a_h), ("M0b", M0b_h), ("Wstk", Wstk_h),
                      ("Gexp", Gexp_h),
                      ("IotaF", np.broadcast_to(
                          np.arange(P, dtype=np.float32), (P, P))),
                      ("M1e", M1e_h), ("M2e", M2e_h)):
        off, rows, cols = wlay[name]
        wfull_h[off:off + rows * cols] = np.asarray(
            arr, np.float32).reshape(-1)

    xd = x_dftb.reshape(E, 144)
    in_maps = []
    for k in range(NCORES):
        c = cores[k]
        blk = xd[k * NB:(k + 1) * NB]
        bf = np.zeros(totf, np.float32)
        def putf(name, arr):
            off, rows, cols = layf[name]
            bf[off:off + rows * cols] = np.asarray(
                arr, np.float32).reshape(-1)
        putf("xdT1", blk[:, 0:P].T)
        putf("xdT2", blk[:, P:144].T)
        putf("radT", c['radT'][:B])
        putf("YshT", c['YshT'][1:9])
        putf("colidx", c['colidx'].T)
        putf("wts", wfull_h)
        bi = np.concatenate([c['srclo'].T, c['srchi'].T, c['sidx'].T], 1)
        in_maps.append({
            "blobf": bf,
            "blobi": np.ascontiguousarray(bi, np.int32),
        })

    import time
    from concourse import bass_utils

    _install_axon_profile_hooks()
    t0 = time.time()
    br = bass_utils.run_bass_kernel_spmd(
        nc, in_maps, core_ids=list(range(NCORES)), trace=True,
        trace_cores=list(range(NCORES)))
    wall_ns = int((time.time() - t0) * 1e9)
    global LAST_EXEC_NS, LAST_TRACE
    # exec_time_ns is the neuron-profiled NEFF execution time on device,
    # max over the 8 cores. Fall back to call wall time (which includes
    # tunnel transfer + jit compile) only if profiling was unavailable.
    LAST_EXEC_NS = br.exec_time_ns if br.exec_time_ns else wall_ns
    LAST_TRACE = br.instructions_and_trace[1] \
        if br.instructions_and_trace else None
    out = np.zeros((E, 1, 9, 1), np.float32)
    for k in range(NCORES):
        out[k * NB:(k + 1) * NB, 0, :, 0] = \
            br.results[k]["outG"].astype(np.float32).T  # [9, NB] f16
    return out

